# revision 26
# baseline (speedup 1.0000x reference)
"""Trainium2 Bass kernel for nn_DecoderLayer (dense transformer decoder layer).

Strategy (8 NeuronCores, full inputs in / full output out):
  - core c handles batch b = c//4 and query-quarter r = c%4 (rows [r*S/4, (r+1)*S/4)).
  - All matmul operands are bf16 (PSUM accumulation fp32); activations are
    kept TRANSPOSED on-chip (x^T [D, n]) so projections run with the
    contraction dim on partitions.
  - K^T / V / Q^T live entirely in SBUF (no HBM round-trip).
  - Self-attention causal masking: the host rotates each core's key order so
    the 4 "diagonal" key tiles sit at fixed slots (last group); keys fully
    below the diagonal get their V rows zeroed at projection time (per-key
    0/1 scale fused into the PSUM evacuation), so only the last exp group
    needs a (core-independent, constant) triangular multiplicative mask.
  - Cross-attention key masking (enc mask) uses the same V-row zeroing.
  - Softmax denominators come free from a ones column appended to V.
  - The single collective: AllGather of x1 (post-LN1) within each 4-core
    batch group, needed because cross-attention K2/V2 are projections of the
    full x1.
  - LayerNorm runs in transposed layout: cross-partition sums via ones-matmul
    on the PE, stats broadcast back to [128, W] via ones-matmul.
"""

import sys

if "/opt/trn_rl_repo" not in sys.path:
    sys.path.insert(0, "/opt/trn_rl_repo")

import numpy as np

P = 128
HD = 64
HD1 = HD + 1
EPS = 1e-5


class Cfg:
    def __init__(self, B=2, S=2048, D=1024, H=16, DFF=4096, use_collective=True,
                 fake_gather=False):
        self.B, self.S, self.D, self.H, self.DFF = B, S, D, H, DFF
        self.fake_gather = fake_gather
        self.W = S // 4            # local query rows per core
        self.DT = D // P           # feature-dim tiles
        self.NT = S // P           # sequence tiles (keys)
        self.FT = DFF // P         # ffn hidden tiles
        self.HP = P // HD          # heads per partition-tile (2)
        self.QT = self.W // P      # key tiles per query quarter (diag tiles)
        self.KTG = min(2, self.NT)    # k-tiles per exp group
        self.NG = self.NT // self.KTG
        self.VCW = min(512, D)        # v-dout chunk width
        self.VCN = D // self.VCW
        self.HPC = self.VCW // HD     # heads per v-chunk
        self.KCW = min(512, self.S)   # k-proj token chunk width
        self.KCN = self.S // self.KCW
        self.G4 = min(4, self.DT)
        self.use_collective = use_collective
        assert D == H * HD
        assert self.W % P == 0 and D % P == 0 and DFF % P == 0 and S % P == 0


class Flags:
    def __init__(self):
        self.qkb1 = self.vb1 = self.ob1 = False
        self.qkb2 = self.vb2 = self.ob2 = False
        self.fb1 = self.fb2 = False
        self.g1 = self.b1 = self.g2 = self.b2 = self.g3 = self.b3 = False
        self.m1 = True      # trg mask active (tril fast path)
        self.m1full = False  # generic (non-tril) trg mask: full mult tiles
        self.zv2 = False    # enc mask active -> zero V2 rows


def _build(nc, tc, cfg, fl):
    import concourse.bass as bass
    import concourse.mybir as mybir
    import concourse.tile as tile  # noqa: F401
    from contextlib import ExitStack

    AF = mybir.ActivationFunctionType
    f32 = mybir.dt.float32
    bf16 = mybir.dt.bfloat16

    B, S, D, H, DFF = cfg.B, cfg.S, cfg.D, cfg.H, cfg.DFF
    W, DT, NT, FT, HP = cfg.W, cfg.DT, cfg.NT, cfg.FT, cfg.HP
    QT, KTG, NG = cfg.QT, cfg.KTG, cfg.NG
    VCW, VCN, HPC = cfg.VCW, cfg.VCN, cfg.HPC
    KCW, KCN, G4 = cfg.KCW, cfg.KCN, cfg.G4

    # ---------------- DRAM parameters ----------------
    def din(name, shape, dt=bf16):
        return nc.dram_tensor(name, shape, dt, kind="ExternalInput").ap()

    xTr = din("xTr", [D, S])          # rotated x^T for this core
    qkvwT1 = din("qkvwT1", [D, 3 * D])
    qkvwT2 = din("qkvwT2", [D, 3 * D])
    owT1 = din("owT1", [D, D])
    owT2 = din("owT2", [D, D])
    w1T = din("w1T", [D, DFF])
    w2T = din("w2T", [DFF, D])
    m1c = din("m1c", [P, QT, W]) if (fl.m1 and not fl.m1full) else None
    m1f = din("m1f", [NT, P, W]) if fl.m1full else None
    zv1 = din("zv1", [P, NT], f32) if fl.m1 and not fl.m1full else None
    zv2d = din("zv2", [P, QT], f32) if fl.zv2 else None
    qkvb1 = din("qkvb1", [3 * D], f32) if fl.qkb1 else None
    qkvb2 = din("qkvb2", [3 * D], f32) if fl.qkb2 else None
    vb1 = din("vb1", [P, D]) if fl.vb1 else None
    vb2 = din("vb2", [P, D]) if fl.vb2 else None
    ob1 = din("ob1", [D], f32) if fl.ob1 else None
    ob2 = din("ob2", [D], f32) if fl.ob2 else None
    fb1d = din("fb1", [DFF], f32) if fl.fb1 else None
    fb2d = din("fb2", [D], f32) if fl.fb2 else None
    lnp = {}
    for nm, use in [("g1", fl.g1), ("b1", fl.b1), ("g2", fl.g2),
                    ("b2", fl.b2), ("g3", fl.g3), ("b3", fl.b3)]:
        lnp[nm] = din(nm, [D], f32) if use else None
    out = nc.dram_tensor("out", [D, W], f32, kind="ExternalOutput").ap()

    es = ExitStack()
    with es:
        dramp = es.enter_context(tc.tile_pool(name="dram", bufs=1, space="DRAM"))
        LKV = D * W + W * H * HD1  # flat K2loc + V2loc staging elements
        if cfg.use_collective:
            kvs = dramp.tile([LKV], bf16)
            agkv = dramp.tile([4 * LKV], bf16)

        const = es.enter_context(tc.tile_pool(name="const", bufs=1))
        ones_p1 = const.tile([P, 1], f32)
        nc.vector.memset(ones_p1[:, :], 1.0)
        ones_1p = const.tile([1, P], f32)
        nc.vector.memset(ones_1p[0:1, :], 1.0)
        ones_col = const.tile([P, HPC, 1], bf16)
        nc.vector.memset(ones_col[:, :, :], 1.0)
        eps_t = const.tile([1, 1], f32)
        nc.vector.memset(eps_t[0:1, :], EPS)

        def ldvec(dram_vec, n_tiles, name):
            """[D]-style f32 vector -> [P, n_tiles] sbuf tile."""
            t = const.tile([P, n_tiles], f32, tag=name)
            nc.sync.dma_start(
                out=t[:, :],
                in_=dram_vec.rearrange("(t p) -> p t", p=P),
            )
            return t

        qkb1sb = ldvec(qkvb1[0 : 2 * D], 2 * DT, "qkb1") if fl.qkb1 else None
        qkb2sb = ldvec(qkvb2[0 : 2 * D], 2 * DT, "qkb2") if fl.qkb2 else None
        ob1sb = ldvec(ob1, DT, "ob1") if fl.ob1 else None
        ob2sb = ldvec(ob2, DT, "ob2") if fl.ob2 else None
        fb1sb = ldvec(fb1d, FT, "fb1") if fl.fb1 else None
        fb2sb = ldvec(fb2d, DT, "fb2") if fl.fb2 else None
        lns = {k: (ldvec(v, DT, "ln" + k) if v is not None else None)
               for k, v in lnp.items()}
        vb1sb = None
        if fl.vb1:
            vb1sb = const.tile([P, D], bf16, tag="vb1")
            nc.sync.dma_start(out=vb1sb[:, :], in_=vb1[:, :])
        vb2sb = None
        if fl.vb2:
            vb2sb = const.tile([P, D], bf16, tag="vb2")
            nc.sync.dma_start(out=vb2sb[:, :], in_=vb2[:, :])
        zv1sb = None
        if zv1 is not None:
            zv1sb = const.tile([P, NT], f32, tag="zv1")
            nc.sync.dma_start(out=zv1sb[:, :], in_=zv1[:, :])
        zv2sb = None
        if zv2d is not None:
            zv2sb = const.tile([P, QT], f32, tag="zv2")
            nc.sync.dma_start(out=zv2sb[:, :], in_=zv2d[:, :])
        m1sb = None
        if m1c is not None:
            m1sb = const.tile([P, QT, W], bf16, tag="m1c")
            nc.sync.dma_start(out=m1sb[:, :, :], in_=m1c[:, :, :])
        m1fsb = None
        if m1f is not None:
            m1fsb = const.tile([P, NT, W], bf16, tag="m1f")
            nc.sync.dma_start(out=m1fsb[:, :, :], in_=m1f.rearrange("n p w -> p n w"))

        # persistent mid tensors
        midp = es.enter_context(tc.tile_pool(name="mid", bufs=1))
        qT = midp.tile([P, DT, W], bf16)      # Q^T local (reused block2)
        aoT = midp.tile([P, DT, W], bf16)     # attention out^T (reused)
        x1T = midp.tile([P, DT, W], bf16)     # x1 local
        x2T = midp.tile([P, DT, W], bf16)     # x2 local
        xlocT = midp.tile([P, DT, W], bf16)   # this core's x quarter (resid 1)
        xf = midp.tile([P, DT, W], f32)       # f32 residual backbone (x1, x2)

        # =========== QKV projection (into SBUF K/V/Q) ===========
        def qkv_phase(xsb, xq, wT, kT, v, qkb, vbsb, zvsb):
            """xsb: [P, DT, Sx] bf16 x^T source (tokens = key order);
            xq: [P, DT, W] bf16 x^T source for this core's queries;
            writes kT [P, DT, Sx] sbuf, v [P, NT_x, H, HD1] sbuf, qT."""
            Sx = xsb.shape[2]
            NTx = Sx // P
            KCWx = min(KCW, Sx)
            KCNx = Sx // KCWx
            with tc.tile_pool(name="qkv_w", bufs=2) as wp, \
                 tc.tile_pool(name="qkv_ps", bufs=2, space="PSUM") as psp:
                # ---- K^T ----
                wall = wp.tile([P, DT, D], bf16, tag="wall")
                nc.sync.dma_start(
                    out=wall[:, :, :],
                    in_=wT[:, D : 2 * D].rearrange("(t p) v -> p t v", p=P),
                )
                for nch in range(KCNx):
                    for dk in range(DT):
                        ps = psp.tile([P, KCWx], f32, tag="kps")
                        for dt in range(DT):
                            nc.tensor.matmul(
                                ps[:, :],
                                lhsT=wall[:, dt, dk * P : (dk + 1) * P],
                                rhs=xsb[:, dt, nch * KCWx : (nch + 1) * KCWx],
                                start=(dt == 0),
                                stop=(dt == DT - 1),
                            )
                        if qkb is not None:
                            nc.scalar.activation(
                                out=kT[:, dk, nch * KCWx : (nch + 1) * KCWx],
                                in_=ps[:, :], func=AF.Identity,
                                bias=qkb[:, DT + dk : DT + dk + 1], scale=1.0,
                            )
                        else:
                            nc.scalar.activation(
                                out=kT[:, dk, nch * KCWx : (nch + 1) * KCWx],
                                in_=ps[:, :], func=AF.Copy,
                            )
                # ---- Q^T local [D, W] ----
                wall = wp.tile([P, DT, D], bf16, tag="wall")
                nc.sync.dma_start(
                    out=wall[:, :, :],
                    in_=wT[:, 0:D].rearrange("(t p) v -> p t v", p=P),
                )
                for dq in range(DT):
                    ps = psp.tile([P, W], f32, tag="qps")
                    for dt in range(DT):
                        nc.tensor.matmul(
                            ps[:, :],
                            lhsT=wall[:, dt, dq * P : (dq + 1) * P],
                            rhs=xq[:, dt, :],
                            start=(dt == 0),
                            stop=(dt == DT - 1),
                        )
                    if qkb is not None:
                        nc.scalar.activation(
                            out=qT[:, dq, :], in_=ps[:, :], func=AF.Identity,
                            bias=qkb[:, dq : dq + 1], scale=1.0,
                        )
                    else:
                        nc.scalar.activation(
                            out=qT[:, dq, :], in_=ps[:, :], func=AF.Copy,
                        )
                # ---- V natural [n, dout] + ones column ----
                wall = wp.tile([P, DT, D], bf16, tag="wall")
                nc.sync.dma_start(
                    out=wall[:, :, :],
                    in_=wT[:, 2 * D : 3 * D].rearrange("(t p) v -> p t v", p=P),
                )
                for nt in range(NTx):
                    for vc in range(VCN):
                        ps = psp.tile([P, VCW], f32, tag="vps")
                        for dt in range(DT):
                            nc.tensor.matmul(
                                ps[:, :],
                                lhsT=xsb[:, dt, nt * P : (nt + 1) * P],
                                rhs=wall[:, dt, vc * VCW : (vc + 1) * VCW],
                                start=(dt == 0),
                                stop=(dt == DT - 1),
                            )
                        dst = v[:, nt, vc * HPC : (vc + 1) * HPC, 0:HD]
                        psv = ps.rearrange("p (h d) -> p h d", d=HD)
                        if vbsb is not None:
                            # bias first, then per-key zeroing (mask applies
                            # to the biased value)
                            nc.scalar.activation(out=dst, in_=psv, func=AF.Copy)
                            nc.vector.tensor_add(
                                dst, dst,
                                vbsb[:, vc * VCW : (vc + 1) * VCW].rearrange(
                                    "p (h d) -> p h d", d=HD),
                            )
                            if zvsb is not None:
                                nc.vector.tensor_scalar_mul(
                                    dst, dst, zvsb[:, nt : nt + 1])
                        elif zvsb is not None:
                            nc.scalar.activation(
                                out=dst, in_=psv, func=AF.Copy,
                                scale=zvsb[:, nt : nt + 1],
                            )
                        else:
                            nc.scalar.activation(out=dst, in_=psv, func=AF.Copy)
                        oc = v[:, nt, vc * HPC : (vc + 1) * HPC, HD:HD1]
                        if zvsb is not None:
                            nc.scalar.activation(
                                out=oc, in_=ones_col[:, :, :], func=AF.Copy,
                                scale=zvsb[:, nt : nt + 1],
                            )
                        else:
                            nc.vector.memset(oc, 1.0)

        # =========== attention phase ===========
        def attn_phase(kT, v, msb, mfull):
            """msb: [P, QT, W] triangular mask on the LAST QT slots (or None);
            mfull: [P, NT, W] generic multiplicative mask (or None).
            Score PSUM is manually double-buffered (two KTG-slot halves of one
            [P, 2*KTG, W] tile) so the next group's matmuls never wait on the
            previous group's exp."""
            NTx = v.shape[1]
            NGx = NTx // KTG
            with tc.tile_pool(name="at_ex", bufs=3) as exp_, \
                 tc.tile_pool(name="at_dn", bufs=2) as dnp, \
                 tc.tile_pool(name="at_ps", bufs=1, space="PSUM") as psp, \
                 tc.tile_pool(name="at_po", bufs=2, space="PSUM") as pop:
                for h in range(H):
                    hh = (h % HP) * HD
                    q_h = qT[hh : hh + HD, h // HP, :]
                    po = pop.tile([P, W], f32, tag="po")
                    psd = psp.tile([P, 2 * KTG, W], f32, tag="sc")

                    def scores(g):
                        ps = psd[:, (g % 2) * KTG : (g % 2) * KTG + KTG, :]
                        for o in range(KTG):
                            kt = g * KTG + o
                            nc.tensor.matmul(
                                ps[:, o, :],
                                lhsT=kT[hh : hh + HD, h // HP,
                                        kt * P : (kt + 1) * P],
                                rhs=q_h,
                                start=True,
                                stop=True,
                            )

                    # software-pipelined: scores(g+1) is issued to the PE
                    # ahead of AV(g), so the PE never sits behind an AV that
                    # is itself waiting on exp(g).
                    scores(0)
                    for g in range(NGx):
                        if g + 1 < NGx:
                            scores(g + 1)
                        ps = psd[:, (g % 2) * KTG : (g % 2) * KTG + KTG, :]
                        ex = exp_.tile([P, KTG, W], bf16, tag="ex")
                        nc.scalar.activation(
                            out=ex[:, :, :], in_=ps[:, :, :], func=AF.Exp,
                            scale=1.0 / float(np.sqrt(HD)),
                        )
                        if mfull is not None:
                            nc.vector.tensor_mul(
                                ex[:, :, :], ex[:, :, :],
                                mfull[:, g * KTG : (g + 1) * KTG, :],
                            )
                        elif msb is not None:
                            # overlap of this group's slots with the diagonal
                            # region [NTx-QT, NTx)
                            lo = max(g * KTG, NTx - QT)
                            hi = (g + 1) * KTG
                            if lo < hi:
                                nc.vector.tensor_mul(
                                    ex[:, lo - g * KTG : KTG, :],
                                    ex[:, lo - g * KTG : KTG, :],
                                    msb[:, lo - (NTx - QT) : hi - (NTx - QT), :],
                                )
                        for o in range(KTG):
                            kt = g * KTG + o
                            nc.tensor.matmul(
                                po[0:HD1, :],
                                lhsT=v[:, kt, h, :],
                                rhs=ex[:, o, :],
                                start=(g == 0 and o == 0),
                                stop=(g == NGx - 1 and o == KTG - 1),
                            )
                    dinv = dnp.tile([1, W], f32, tag="dinv")
                    nc.vector.reciprocal(dinv[0:1, :], po[HD:HD1, :])
                    dinvb = dnp.tile([HD, W], f32, tag="dinvb")
                    nc.gpsimd.partition_broadcast(
                        dinvb[0:HD, :], dinv[0:1, :], channels=HD
                    )
                    nc.vector.tensor_mul(
                        aoT[hh : hh + HD, h // HP, :],
                        po[0:HD, :],
                        dinvb[0:HD, :],
                    )

        # =========== layernorm (transposed layout, f32 internals) ===========
        def ln_t(pre, out_bf, out_f, g_sb, b_sb, lpp, lp):
            """pre: [P, DT, W] f32 sbuf; out_bf bf16 (or None), out_f f32
            (or None; at least one)."""
            acc = lp.tile([P, W], f32, tag="lnacc")
            nc.vector.tensor_add(acc[:, :], pre[:, 0, :], pre[:, 1, :])
            for d in range(2, DT):
                nc.vector.tensor_add(acc[:, :], acc[:, :], pre[:, d, :])
            sqa = lp.tile([P, W], f32, tag="lnsqa")
            nc.scalar.square(sqa[:, :], pre[:, 0, :])
            for d in range(1, DT):
                sqt = lp.tile([P, W], f32, tag="lnsqt")
                nc.scalar.square(sqt[:, :], pre[:, d, :])
                nc.vector.tensor_add(sqa[:, :], sqa[:, :], sqt[:, :])
            sums = lpp.tile([1, W], f32, tag="lnsums")
            nc.tensor.matmul(sums[0:1, :], lhsT=ones_p1[:, :],
                             rhs=acc[:, :], start=True, stop=True)
            sqs = lpp.tile([1, W], f32, tag="lnsqs")
            nc.tensor.matmul(sqs[0:1, :], lhsT=ones_p1[:, :],
                             rhs=sqa[:, :], start=True, stop=True)
            mu = lp.tile([1, W], f32, tag="lnmu")
            nc.vector.tensor_scalar_mul(mu[0:1, :], sums[0:1, :], 1.0 / D)
            ex2 = lp.tile([1, W], f32, tag="lnex2")
            nc.vector.tensor_scalar_mul(ex2[0:1, :], sqs[0:1, :], 1.0 / D)
            mu2 = lp.tile([1, W], f32, tag="lnmu2")
            nc.scalar.square(mu2[0:1, :], mu[0:1, :])
            var = lp.tile([1, W], f32, tag="lnvar")
            nc.vector.tensor_sub(var[0:1, :], ex2[0:1, :], mu2[0:1, :])
            sd = lp.tile([1, W], f32, tag="lnsd")
            nc.scalar.activation(out=sd[0:1, :], in_=var[0:1, :], func=AF.Sqrt,
                                 bias=eps_t[0:1, :], scale=1.0)
            rstd = lp.tile([1, W], f32, tag="lnrstd")
            nc.vector.reciprocal(rstd[0:1, :], sd[0:1, :])
            mub = lpp.tile([P, W], f32, tag="lnmub")
            nc.tensor.matmul(mub[:, :], lhsT=ones_1p[0:1, :],
                             rhs=mu[0:1, :], start=True, stop=True)
            rstdb = lpp.tile([P, W], f32, tag="lnrstdb")
            nc.tensor.matmul(rstdb[:, :], lhsT=ones_1p[0:1, :],
                             rhs=rstd[0:1, :], start=True, stop=True)
            mubs = lp.tile([P, W], f32, tag="lnmubs")
            nc.vector.tensor_copy(mubs[:, :], mub[:, :])
            rstdbs = lp.tile([P, W], f32, tag="lnrstdbs")
            nc.vector.tensor_copy(rstdbs[:, :], rstdb[:, :])
            for d in range(DT):
                t1 = lp.tile([P, W], f32, tag="lnt1")
                nc.vector.tensor_sub(t1[:, :], pre[:, d, :], mubs[:, :])
                of = out_f[:, d, :] if out_f is not None else None
                if of is not None:
                    nc.vector.tensor_mul(of, t1[:, :], rstdbs[:, :])
                    if g_sb is not None:
                        nc.vector.tensor_scalar_mul(of, of, g_sb[:, d : d + 1])
                    if b_sb is not None:
                        nc.vector.tensor_scalar_add(of, of, b_sb[:, d : d + 1])
                    if out_bf is not None:
                        nc.vector.tensor_copy(out_bf[:, d, :], of)
                else:
                    ob = out_bf[:, d, :]
                    nc.vector.tensor_mul(ob, t1[:, :], rstdbs[:, :])
                    if g_sb is not None:
                        nc.vector.tensor_scalar_mul(ob, ob, g_sb[:, d : d + 1])
                    if b_sb is not None:
                        nc.vector.tensor_scalar_add(ob, ob, b_sb[:, d : d + 1])

        # =========== out-projection + residual + LN ===========
        def proj_resid_ln(owT, obsb, residT, g_sb, b_sb, out_bf, out_f):
            with tc.tile_pool(name="pr_w", bufs=2) as wp, \
                 tc.tile_pool(name="pr_t", bufs=2) as lp, \
                 tc.tile_pool(name="pr_pre", bufs=1) as prep, \
                 tc.tile_pool(name="pr_ps", bufs=2, space="PSUM") as psp, \
                 tc.tile_pool(name="pr_lnps", bufs=1, space="PSUM") as lpp:
                pre = prep.tile([P, DT, W], f32, tag="pre")
                for dg in range(DT // G4):
                    wsl = wp.tile([P, DT, G4 * P], bf16, tag="prw")
                    nc.sync.dma_start(
                        out=wsl[:, :, :],
                        in_=owT[:, dg * G4 * P : (dg + 1) * G4 * P]
                        .rearrange("(t p) v -> p t v", p=P),
                    )
                    for j in range(G4):
                        d = dg * G4 + j
                        ps = psp.tile([P, W], f32, tag="prps")
                        for dt in range(DT):
                            nc.tensor.matmul(
                                ps[:, :], lhsT=wsl[:, dt, j * P : (j + 1) * P],
                                rhs=aoT[:, dt, :],
                                start=(dt == 0), stop=(dt == DT - 1),
                            )
                        if obsb is not None:
                            tmp = lp.tile([P, W], f32, tag="prtmp")
                            nc.scalar.activation(out=tmp[:, :], in_=ps[:, :],
                                                 func=AF.Identity,
                                                 bias=obsb[:, d : d + 1], scale=1.0)
                            nc.vector.tensor_add(pre[:, d, :], tmp[:, :],
                                                 residT[:, d, :])
                        else:
                            nc.vector.tensor_add(pre[:, d, :], ps[:, :],
                                                 residT[:, d, :])
                ln_t(pre, out_bf, out_f, g_sb, b_sb, lpp, lp)

        # ================= block 1: self-attention =================
        with tc.tile_pool(name="kv1", bufs=1) as kvp1:
            kT1 = kvp1.tile([P, DT, S], bf16)
            v1 = kvp1.tile([P, NT, H, HD1], bf16)
            with tc.tile_pool(name="xs1", bufs=1) as xsp1:
                xs = xsp1.tile([P, DT, S], bf16)
                nc.sync.dma_start(out=xs[:, :, :],
                                  in_=xTr.rearrange("(t p) s -> p t s", p=P))
                # the host always rotates key order so this core's quarter
                # sits in the last QT slots (uniform across cores); any mask
                # data is supplied in rotated coordinates.
                xq_off = S - W
                xloc = xs[:, :, xq_off : xq_off + W]
                nc.vector.tensor_copy(xlocT[:, :, :], xloc)
                qkv_phase(xs, xloc, qkvwT1, kT1, v1, qkb1sb, vb1sb, zv1sb)
            attn_phase(kT1, v1, m1sb, m1fsb)
        proj_resid_ln(owT1, ob1sb, xlocT, lns["g1"], lns["b1"], x1T, xf)

        # ---- local K2/V2/Q2 from x1, then all-gather K2|V2 in group ----
        assert cfg.use_collective
        with tc.tile_pool(name="kvloc", bufs=1) as kvlp:
            klocT = kvlp.tile([P, DT, W], bf16)
            vloc = kvlp.tile([P, QT, H, HD1], bf16)
            qkv_phase(x1T, x1T, qkvwT2, klocT, vloc, qkb2sb, vb2sb, zv2sb)
            nc.sync.dma_start(
                out=kvs[0 : D * W].rearrange("(t p w) -> p t w", p=P, w=W),
                in_=klocT[:, :, :],
            )
            nc.sync.dma_start(
                out=kvs[D * W : LKV].rearrange("(q p h d) -> p q h d",
                                               p=P, h=H, d=HD1),
                in_=vloc[:, :, :, :],
            )
        if cfg.fake_gather:
            for g in range(4):
                nc.sync.dma_start(out=agkv[g * LKV : (g + 1) * LKV],
                                  in_=kvs[:])
        else:
            nc.gpsimd.collective_compute(
                "AllGather",
                bass.mybir.AluOpType.bypass,
                replica_groups=[[0, 1, 2, 3], [4, 5, 6, 7]],
                ins=[kvs[:]],
                outs=[agkv[:]],
            )

        # ================= block 2: cross-attention =================
        with tc.tile_pool(name="kv2", bufs=1) as kvp2:
            kT2 = kvp2.tile([P, DT, S], bf16)
            v2 = kvp2.tile([P, NT, H, HD1], bf16)
            for g in range(4):
                nc.sync.dma_start(
                    out=kT2[:, :, g * W : (g + 1) * W],
                    in_=agkv[g * LKV : g * LKV + D * W]
                    .rearrange("(t p w) -> p t w", p=P, w=W),
                )
                nc.sync.dma_start(
                    out=v2[:, g * QT : (g + 1) * QT, :, :],
                    in_=agkv[g * LKV + D * W : (g + 1) * LKV]
                    .rearrange("(q p h d) -> p q h d", p=P, h=H, d=HD1),
                )
            attn_phase(kT2, v2, None, None)
        proj_resid_ln(owT2, ob2sb, xf, lns["g2"], lns["b2"], x2T, xf)

        # ================= FFN =================
        with tc.tile_pool(name="ffh", bufs=1) as fhp, \
             tc.tile_pool(name="ffw", bufs=2) as wp, \
             tc.tile_pool(name="fft", bufs=1) as lp, \
             tc.tile_pool(name="ffpre", bufs=1) as prep:
            hT = fhp.tile([P, FT, W], bf16)
            with tc.tile_pool(name="ffps1", bufs=2, space="PSUM") as psp:
                for fg in range(FT // G4):
                    wsl = wp.tile([P, DT, G4 * P], bf16, tag="f1w")
                    nc.sync.dma_start(
                        out=wsl[:, :, :],
                        in_=w1T[:, fg * G4 * P : (fg + 1) * G4 * P]
                        .rearrange("(t p) v -> p t v", p=P),
                    )
                    for j in range(G4):
                        f = fg * G4 + j
                        ps = psp.tile([P, W], f32, tag="f1ps")
                        for dt in range(DT):
                            nc.tensor.matmul(
                                ps[:, :], lhsT=wsl[:, dt, j * P : (j + 1) * P],
                                rhs=x2T[:, dt, :],
                                start=(dt == 0), stop=(dt == DT - 1),
                            )
                        if fb1sb is not None:
                            nc.scalar.activation(out=hT[:, f, :], in_=ps[:, :],
                                                 func=AF.Relu,
                                                 bias=fb1sb[:, f : f + 1], scale=1.0)
                        else:
                            nc.scalar.activation(out=hT[:, f, :], in_=ps[:, :],
                                                 func=AF.Relu)
            pre = prep.tile([P, DT, W], f32, tag="ffpre")
            FTC = min(8, FT)  # w2 staging chunk (ft tiles per DMA)
            with tc.tile_pool(name="ffps2", bufs=1, space="PSUM") as psq, \
                 tc.tile_pool(name="fflnps", bufs=1, space="PSUM") as lpp:
                for dg in range(DT // G4):
                    ps4 = []
                    for j in range(G4):
                        ps4j = psq.tile([P, W], f32, tag="f2ps%d" % j)
                        ps4.append(ps4j)
                    for fc in range(FT // FTC):
                        w2sl = wp.tile([P, FTC, G4 * P], bf16, tag="f2w")
                        nc.sync.dma_start(
                            out=w2sl[:, :, :],
                            in_=w2T[fc * FTC * P : (fc + 1) * FTC * P,
                                    dg * G4 * P : (dg + 1) * G4 * P]
                            .rearrange("(t p) v -> p t v", p=P),
                        )
                        for fo in range(FTC):
                            ft = fc * FTC + fo
                            for j in range(G4):
                                nc.tensor.matmul(
                                    ps4[j][:, :],
                                    lhsT=w2sl[:, fo, j * P : (j + 1) * P],
                                    rhs=hT[:, ft, :],
                                    start=(ft == 0), stop=(ft == FT - 1),
                                )
                    for j in range(G4):
                        d = dg * G4 + j
                        if fb2sb is not None:
                            tmp = lp.tile([P, W], f32, tag="f2tmp")
                            nc.scalar.activation(out=tmp[:, :], in_=ps4[j][:, :],
                                                 func=AF.Identity,
                                                 bias=fb2sb[:, d : d + 1], scale=1.0)
                            nc.vector.tensor_add(pre[:, d, :], tmp[:, :],
                                                 xf[:, d, :])
                        else:
                            nc.vector.tensor_add(pre[:, d, :], ps4[j][:, :],
                                                 xf[:, d, :])
                outp = prep.tile([P, DT, W], f32, tag="ffout")
                ln_t(pre, None, outp, lns["g3"], lns["b3"], lpp, lp)
                nc.sync.dma_start(
                    out=out.rearrange("(t p) w -> p t w", p=P),
                    in_=outp[:, :, :])


def make_program(cfg, fl):
    from concourse import bacc
    import concourse.tile as tile

    nc = bacc.Bacc("TRN2", target_bir_lowering=False, debug=False,
                   num_devices=8)
    with tile.TileContext(nc) as tc:
        with nc.allow_low_precision(reason="bf16 kernel, rel-err gate 2e-2"):
            _build(nc, tc, cfg, fl)
    nc.compile()
    return nc


def prep_inputs(inputs, cfg):
    """Host-side data prep. Returns (in_maps, fl)."""
    import ml_dtypes

    bf = ml_dtypes.bfloat16
    B, S, D, H, DFF, W, NT, QT = (cfg.B, cfg.S, cfg.D, cfg.H, cfg.DFF,
                                  cfg.W, cfg.NT, cfg.QT)
    f = np.float32
    x = np.asarray(inputs["x"], f)
    enc = np.asarray(inputs["enc_out"])
    trg = np.asarray(inputs["trg_mask"])
    fl = Flags()
    fl.qkb1 = bool(np.any(inputs["qkv_b1"]))
    fl.qkb2 = bool(np.any(inputs["qkv_b2"]))
    fl.vb1 = bool(np.any(np.asarray(inputs["qkv_b1"])[2 * D :]))
    fl.vb2 = bool(np.any(np.asarray(inputs["qkv_b2"])[2 * D :]))
    fl.ob1 = bool(np.any(inputs["out_b1"]))
    fl.ob2 = bool(np.any(inputs["out_b2"]))
    fl.fb1 = bool(np.any(inputs["ff_b1"]))
    fl.fb2 = bool(np.any(inputs["ff_b2"]))
    fl.g1 = not bool(np.all(np.asarray(inputs["ln1_g"]) == 1))
    fl.b1 = bool(np.any(inputs["ln1_b"]))
    fl.g2 = not bool(np.all(np.asarray(inputs["ln2_g"]) == 1))
    fl.b2 = bool(np.any(inputs["ln2_b"]))
    fl.g3 = not bool(np.all(np.asarray(inputs["ln3_g"]) == 1))
    fl.b3 = bool(np.any(inputs["ln3_b"]))
    fl.m1 = not bool(np.all(trg != 0))
    tril = np.tril(np.ones((S, S), np.int32))
    is_tril = (trg.shape[0] == 1 and
               bool(np.array_equal((trg[0, 0] != 0).astype(np.int32), tril)))
    fl.m1full = fl.m1 and not is_tril
    fl.zv2 = bool(np.any(enc == 0))

    def bcast(a):
        return np.ascontiguousarray(np.asarray(a, f).T.astype(bf))

    shared = {
        "qkvwT1": bcast(inputs["qkv_w1"]),
        "qkvwT2": bcast(inputs["qkv_w2"]),
        "owT1": bcast(inputs["out_w1"]),
        "owT2": bcast(inputs["out_w2"]),
        "w1T": bcast(inputs["ff_w1"]),
        "w2T": bcast(inputs["ff_w2"]),
    }
    if fl.qkb1:
        shared["qkvb1"] = np.asarray(inputs["qkv_b1"], f)
    if fl.qkb2:
        shared["qkvb2"] = np.asarray(inputs["qkv_b2"], f)
    if fl.vb1:
        shared["vb1"] = np.broadcast_to(
            np.asarray(inputs["qkv_b1"], f)[2 * D :], (P, D)).astype(bf)
    if fl.vb2:
        shared["vb2"] = np.broadcast_to(
            np.asarray(inputs["qkv_b2"], f)[2 * D :], (P, D)).astype(bf)
    if fl.ob1:
        shared["ob1"] = np.asarray(inputs["out_b1"], f)
    if fl.ob2:
        shared["ob2"] = np.asarray(inputs["out_b2"], f)
    if fl.fb1:
        shared["fb1"] = np.asarray(inputs["ff_b1"], f)
    if fl.fb2:
        shared["fb2"] = np.asarray(inputs["ff_b2"], f)
    for nm, key, use in [("g1", "ln1_g", fl.g1), ("b1", "ln1_b", fl.b1),
                         ("g2", "ln2_g", fl.g2), ("b2", "ln2_b", fl.b2),
                         ("g3", "ln3_g", fl.g3), ("b3", "ln3_b", fl.b3)]:
        if use:
            shared[nm] = np.asarray(inputs[key], f)
    if fl.m1 and not fl.m1full:
        # constant triangular mask for the last QT slots, same on all cores:
        # m1c[k', s, q'] = 1 if s*P + k' <= q'
        kk = np.arange(P)[:, None, None]
        ss = np.arange(QT)[None, :, None]
        qq = np.arange(W)[None, None, :]
        shared["m1c"] = ((ss * P + kk) <= qq).astype(bf)

    xTb = [np.ascontiguousarray(x[b].T).astype(bf) for b in range(B)]
    in_maps = []
    for c in range(8):
        b, r = c // 4, c % 4
        m = dict(shared)
        # rotate key tiles: slot t holds physical tile p(t) = (t+(r+1)*QT)%NT
        perm = [(t + (r + 1) * QT) % NT for t in range(NT)]
        xt = xTb[b].reshape(D, NT, P)
        m["xTr"] = np.ascontiguousarray(
            xt[:, perm, :].reshape(D, S))
        if fl.m1 and not fl.m1full:
            zv = np.zeros((P, NT), f)
            for t in range(NT):
                if perm[t] < (r + 1) * QT:
                    zv[:, t] = 1
            m["zv1"] = zv
        if fl.m1full:
            # generic multiplicative mask in rotated key coordinates
            tb = trg[b] if trg.shape[0] == B else trg[0]
            blk = (tb[0, r * W : (r + 1) * W, :] != 0).astype(f)  # [W, S](q,k)
            mk = blk.T.reshape(NT, P, W)  # [kt, k', q]
            m["m1f"] = np.ascontiguousarray(mk[perm]).astype(bf)
        if fl.zv2:
            # this core's own quarter only (keys it contributes to the
            # gathered V2) -- applied before the gather, so consumers see
            # already-zeroed rows
            eb = (np.asarray(enc)[b, 0, 0, r * W : (r + 1) * W] != 0).astype(f)
            m["zv2"] = np.ascontiguousarray(
                eb.reshape(W // P, P).T).astype(f)
        in_maps.append(m)
    return in_maps, fl


def kernel_with_results(**inputs):
    from concourse.bass_utils import run_bass_kernel_spmd

    cfg = Cfg()
    x = np.asarray(inputs["x"])
    assert x.shape == (cfg.B, cfg.S, cfg.D), x.shape
    in_maps, fl = prep_inputs(inputs, cfg)
    nc = make_program(cfg, fl)
    res = run_bass_kernel_spmd(nc, in_maps, list(range(8)))
    y = np.empty((cfg.B, cfg.S, cfg.D), np.float32)
    for c in range(8):
        b, r = c // 4, c % 4
        y[b, r * cfg.W : (r + 1) * cfg.W, :] = res.results[c]["out"].T
    return y, res


def kernel(**inputs):
    return kernel_with_results(**inputs)[0]


# revision 27
# speedup vs baseline: 1.3133x; 1.3133x over previous
"""Trainium2 Bass kernel for nn_DecoderLayer (dense transformer decoder layer).

Strategy (8 NeuronCores, full inputs in / full output out):
  - core c handles batch b = c//4 and query-quarter r = c%4 (rows [r*S/4, (r+1)*S/4)).
  - All matmul operands are bf16 (PSUM accumulation fp32); activations are
    kept TRANSPOSED on-chip (x^T [D, n]) so projections run with the
    contraction dim on partitions.
  - K^T / V / Q^T live entirely in SBUF (no HBM round-trip).
  - Self-attention causal masking: the host rotates each core's key order so
    the 4 "diagonal" key tiles sit at fixed slots (last group); keys fully
    below the diagonal get their V rows zeroed at projection time (per-key
    0/1 scale fused into the PSUM evacuation), so only the last exp group
    needs a (core-independent, constant) triangular multiplicative mask.
  - Cross-attention key masking (enc mask) uses the same V-row zeroing.
  - Softmax denominators come free from a ones column appended to V.
  - The single collective: AllGather of x1 (post-LN1) within each 4-core
    batch group, needed because cross-attention K2/V2 are projections of the
    full x1.
  - LayerNorm runs in transposed layout: cross-partition sums via ones-matmul
    on the PE, stats broadcast back to [128, W] via ones-matmul.
"""

import sys

if "/opt/trn_rl_repo" not in sys.path:
    sys.path.insert(0, "/opt/trn_rl_repo")

import numpy as np

P = 128
HD = 64
HD1 = HD + 1
EPS = 1e-5


class Cfg:
    def __init__(self, B=2, S=2048, D=1024, H=16, DFF=4096, use_collective=True,
                 fake_gather=False):
        self.B, self.S, self.D, self.H, self.DFF = B, S, D, H, DFF
        self.fake_gather = fake_gather
        self.W = S // 4            # local query rows per core
        self.DT = D // P           # feature-dim tiles
        self.NT = S // P           # sequence tiles (keys)
        self.FT = DFF // P         # ffn hidden tiles
        self.HP = P // HD          # heads per partition-tile (2)
        self.QT = self.W // P      # key tiles per query quarter (diag tiles)
        self.KTG = min(2, self.NT)    # k-tiles per exp group
        self.NG = self.NT // self.KTG
        self.VCW = min(512, D)        # v-dout chunk width
        self.VCN = D // self.VCW
        self.HPC = self.VCW // HD     # heads per v-chunk
        self.KCW = min(512, self.S)   # k-proj token chunk width
        self.KCN = self.S // self.KCW
        self.G4 = min(4, self.DT)
        self.use_collective = use_collective
        assert D == H * HD
        assert self.W % P == 0 and D % P == 0 and DFF % P == 0 and S % P == 0


class Flags:
    def __init__(self):
        self.qkb1 = self.vb1 = self.ob1 = False
        self.qkb2 = self.vb2 = self.ob2 = False
        self.fb1 = self.fb2 = False
        self.g1 = self.b1 = self.g2 = self.b2 = self.g3 = self.b3 = False
        self.m1 = True      # trg mask active (tril fast path)
        self.m1full = False  # generic (non-tril) trg mask: full mult tiles
        self.zv2 = False    # enc mask active -> zero V2 rows


def _build(nc, tc, cfg, fl):
    import concourse.bass as bass
    import concourse.mybir as mybir
    import concourse.tile as tile  # noqa: F401
    from contextlib import ExitStack

    AF = mybir.ActivationFunctionType
    f32 = mybir.dt.float32
    bf16 = mybir.dt.bfloat16

    B, S, D, H, DFF = cfg.B, cfg.S, cfg.D, cfg.H, cfg.DFF
    W, DT, NT, FT, HP = cfg.W, cfg.DT, cfg.NT, cfg.FT, cfg.HP
    QT, KTG, NG = cfg.QT, cfg.KTG, cfg.NG
    VCW, VCN, HPC = cfg.VCW, cfg.VCN, cfg.HPC
    KCW, KCN, G4 = cfg.KCW, cfg.KCN, cfg.G4

    # ---------------- DRAM parameters ----------------
    def din(name, shape, dt=bf16):
        return nc.dram_tensor(name, shape, dt, kind="ExternalInput").ap()

    xTr = din("xTr", [D, S])          # rotated x^T for this core
    qkvwT1 = din("qkvwT1", [D, 3 * D])
    qkvwT2 = din("qkvwT2", [D, 3 * D])
    owT1 = din("owT1", [D, D])
    owT2 = din("owT2", [D, D])
    w1T = din("w1T", [D, DFF])
    w2T = din("w2T", [DFF, D])
    m1c = din("m1c", [P, QT, W]) if (fl.m1 and not fl.m1full) else None
    m1f = din("m1f", [NT, P, W]) if fl.m1full else None
    zv1 = din("zv1", [P, NT], f32) if fl.m1 and not fl.m1full else None
    zv2d = din("zv2", [P, QT], f32) if fl.zv2 else None
    qkvb1 = din("qkvb1", [3 * D], f32) if fl.qkb1 else None
    qkvb2 = din("qkvb2", [3 * D], f32) if fl.qkb2 else None
    vb1 = din("vb1", [P, D]) if fl.vb1 else None
    vb2 = din("vb2", [P, D]) if fl.vb2 else None
    ob1 = din("ob1", [D], f32) if fl.ob1 else None
    ob2 = din("ob2", [D], f32) if fl.ob2 else None
    fb1d = din("fb1", [DFF], f32) if fl.fb1 else None
    fb2d = din("fb2", [D], f32) if fl.fb2 else None
    lnp = {}
    for nm, use in [("g1", fl.g1), ("b1", fl.b1), ("g2", fl.g2),
                    ("b2", fl.b2), ("g3", fl.g3), ("b3", fl.b3)]:
        lnp[nm] = din(nm, [D], f32) if use else None
    out = nc.dram_tensor("out", [D, W], f32, kind="ExternalOutput").ap()

    es = ExitStack()
    with es:
        dramp = es.enter_context(tc.tile_pool(name="dram", bufs=1, space="DRAM"))
        LKV = D * W + W * H * HD1  # flat K2loc + V2loc staging elements
        if cfg.use_collective:
            kvs = dramp.tile([LKV], bf16)
            agkv = dramp.tile([4 * LKV], bf16)

        const = es.enter_context(tc.tile_pool(name="const", bufs=1))
        ones_p1 = const.tile([P, 1], f32)
        nc.vector.memset(ones_p1[:, :], 1.0)
        ones_1p = const.tile([1, P], f32)
        nc.vector.memset(ones_1p[0:1, :], 1.0)
        ones_col = const.tile([P, HPC, 1], bf16)
        nc.vector.memset(ones_col[:, :, :], 1.0)
        eps_t = const.tile([1, 1], f32)
        nc.vector.memset(eps_t[0:1, :], EPS)

        def ldvec(dram_vec, n_tiles, name):
            """[D]-style f32 vector -> [P, n_tiles] sbuf tile."""
            t = const.tile([P, n_tiles], f32, tag=name)
            nc.sync.dma_start(
                out=t[:, :],
                in_=dram_vec.rearrange("(t p) -> p t", p=P),
            )
            return t

        qkb1sb = ldvec(qkvb1[0 : 2 * D], 2 * DT, "qkb1") if fl.qkb1 else None
        qkb2sb = ldvec(qkvb2[0 : 2 * D], 2 * DT, "qkb2") if fl.qkb2 else None
        ob1sb = ldvec(ob1, DT, "ob1") if fl.ob1 else None
        ob2sb = ldvec(ob2, DT, "ob2") if fl.ob2 else None
        fb1sb = ldvec(fb1d, FT, "fb1") if fl.fb1 else None
        fb2sb = ldvec(fb2d, DT, "fb2") if fl.fb2 else None
        lns = {k: (ldvec(v, DT, "ln" + k) if v is not None else None)
               for k, v in lnp.items()}
        vb1sb = None
        if fl.vb1:
            vb1sb = const.tile([P, D], bf16, tag="vb1")
            nc.sync.dma_start(out=vb1sb[:, :], in_=vb1[:, :])
        vb2sb = None
        if fl.vb2:
            vb2sb = const.tile([P, D], bf16, tag="vb2")
            nc.sync.dma_start(out=vb2sb[:, :], in_=vb2[:, :])
        zv1sb = None
        if zv1 is not None:
            zv1sb = const.tile([P, NT], f32, tag="zv1")
            nc.sync.dma_start(out=zv1sb[:, :], in_=zv1[:, :])
        zv2sb = None
        if zv2d is not None:
            zv2sb = const.tile([P, QT], f32, tag="zv2")
            nc.sync.dma_start(out=zv2sb[:, :], in_=zv2d[:, :])
        m1sb = None
        if m1c is not None:
            m1sb = const.tile([P, QT, W], bf16, tag="m1c")
            nc.sync.dma_start(out=m1sb[:, :, :], in_=m1c[:, :, :])
        m1fsb = None
        if m1f is not None:
            m1fsb = const.tile([P, NT, W], bf16, tag="m1f")
            nc.sync.dma_start(out=m1fsb[:, :, :], in_=m1f.rearrange("n p w -> p n w"))

        # persistent mid tensors
        midp = es.enter_context(tc.tile_pool(name="mid", bufs=1))
        qT = midp.tile([P, DT, W], bf16)      # Q^T local (reused block2)
        aoT = midp.tile([P, DT, W], bf16)     # attention out^T (reused)
        x1T = midp.tile([P, DT, W], bf16)     # x1 local
        x2T = midp.tile([P, DT, W], bf16)     # x2 local
        xlocT = midp.tile([P, DT, W], bf16)   # this core's x quarter (resid 1)
        xf = midp.tile([P, DT, W], f32)       # f32 residual backbone (x1, x2)

        # =========== QKV projection (into SBUF K/V/Q) ===========
        def qkv_phase(xsb, xq, wT, kT, v, qkb, vbsb, zvsb):
            """xsb: [P, DT, Sx] bf16 x^T source (tokens = key order);
            xq: [P, DT, W] bf16 x^T source for this core's queries;
            writes kT [P, DT, Sx] sbuf, v [P, NT_x, H, HD1] sbuf, qT."""
            Sx = xsb.shape[2]
            NTx = Sx // P
            KCWx = min(KCW, Sx)
            KCNx = Sx // KCWx
            with tc.tile_pool(name="qkv_w", bufs=2) as wp, \
                 tc.tile_pool(name="qkv_ps", bufs=2, space="PSUM") as psp:
                # ---- K^T ----
                wall = wp.tile([P, DT, D], bf16, tag="wall")
                nc.sync.dma_start(
                    out=wall[:, :, :],
                    in_=wT[:, D : 2 * D].rearrange("(t p) v -> p t v", p=P),
                )
                for nch in range(KCNx):
                    for dk in range(DT):
                        ps = psp.tile([P, KCWx], f32, tag="kps")
                        for dt in range(DT):
                            nc.tensor.matmul(
                                ps[:, :],
                                lhsT=wall[:, dt, dk * P : (dk + 1) * P],
                                rhs=xsb[:, dt, nch * KCWx : (nch + 1) * KCWx],
                                start=(dt == 0),
                                stop=(dt == DT - 1),
                            )
                        if qkb is not None:
                            nc.scalar.activation(
                                out=kT[:, dk, nch * KCWx : (nch + 1) * KCWx],
                                in_=ps[:, :], func=AF.Identity,
                                bias=qkb[:, DT + dk : DT + dk + 1], scale=1.0,
                            )
                        else:
                            nc.scalar.activation(
                                out=kT[:, dk, nch * KCWx : (nch + 1) * KCWx],
                                in_=ps[:, :], func=AF.Copy,
                            )
                # ---- Q^T local [D, W] ----
                wall = wp.tile([P, DT, D], bf16, tag="wall")
                nc.sync.dma_start(
                    out=wall[:, :, :],
                    in_=wT[:, 0:D].rearrange("(t p) v -> p t v", p=P),
                )
                for dq in range(DT):
                    ps = psp.tile([P, W], f32, tag="qps")
                    for dt in range(DT):
                        nc.tensor.matmul(
                            ps[:, :],
                            lhsT=wall[:, dt, dq * P : (dq + 1) * P],
                            rhs=xq[:, dt, :],
                            start=(dt == 0),
                            stop=(dt == DT - 1),
                        )
                    if qkb is not None:
                        nc.scalar.activation(
                            out=qT[:, dq, :], in_=ps[:, :], func=AF.Identity,
                            bias=qkb[:, dq : dq + 1], scale=1.0,
                        )
                    else:
                        nc.scalar.activation(
                            out=qT[:, dq, :], in_=ps[:, :], func=AF.Copy,
                        )
                # ---- V natural [n, dout] + ones column ----
                wall = wp.tile([P, DT, D], bf16, tag="wall")
                nc.sync.dma_start(
                    out=wall[:, :, :],
                    in_=wT[:, 2 * D : 3 * D].rearrange("(t p) v -> p t v", p=P),
                )
                for nt in range(NTx):
                    for vc in range(VCN):
                        ps = psp.tile([P, VCW], f32, tag="vps")
                        for dt in range(DT):
                            nc.tensor.matmul(
                                ps[:, :],
                                lhsT=xsb[:, dt, nt * P : (nt + 1) * P],
                                rhs=wall[:, dt, vc * VCW : (vc + 1) * VCW],
                                start=(dt == 0),
                                stop=(dt == DT - 1),
                            )
                        dst = v[:, nt, vc * HPC : (vc + 1) * HPC, 0:HD]
                        psv = ps.rearrange("p (h d) -> p h d", d=HD)
                        if vbsb is not None:
                            # bias first, then per-key zeroing (mask applies
                            # to the biased value)
                            nc.scalar.activation(out=dst, in_=psv, func=AF.Copy)
                            nc.vector.tensor_add(
                                dst, dst,
                                vbsb[:, vc * VCW : (vc + 1) * VCW].rearrange(
                                    "p (h d) -> p h d", d=HD),
                            )
                            if zvsb is not None:
                                nc.vector.tensor_scalar_mul(
                                    dst, dst, zvsb[:, nt : nt + 1])
                        elif zvsb is not None:
                            nc.scalar.activation(
                                out=dst, in_=psv, func=AF.Copy,
                                scale=zvsb[:, nt : nt + 1],
                            )
                        else:
                            nc.scalar.activation(out=dst, in_=psv, func=AF.Copy)
                        oc = v[:, nt, vc * HPC : (vc + 1) * HPC, HD:HD1]
                        if zvsb is not None:
                            nc.scalar.activation(
                                out=oc, in_=ones_col[:, :, :], func=AF.Copy,
                                scale=zvsb[:, nt : nt + 1],
                            )
                        else:
                            nc.vector.memset(oc, 1.0)

        # =========== attention phase ===========
        def attn_phase(kT, v, msb, mfull):
            """msb: [P, QT, W] triangular mask on the LAST QT slots (or None);
            mfull: [P, NT, W] generic multiplicative mask (or None).
            Score PSUM is manually double-buffered (two KTG-slot halves of one
            [P, 2*KTG, W] tile) so the next group's matmuls never wait on the
            previous group's exp."""
            NTx = v.shape[1]
            NGx = NTx // KTG
            with tc.tile_pool(name="at_ex", bufs=3) as exp_, \
                 tc.tile_pool(name="at_dn", bufs=2) as dnp, \
                 tc.tile_pool(name="at_ps", bufs=2, space="PSUM") as psp, \
                 tc.tile_pool(name="at_po", bufs=2, space="PSUM") as pop:
                for h in range(H):
                    hh = (h % HP) * HD
                    q_h = qT[hh : hh + HD, h // HP, :]
                    po = pop.tile([P, W], f32, tag="po")
                    pstiles = {}

                    def scores(g):
                        ps = psp.tile([P, KTG, W], f32, tag="sc")
                        pstiles[g] = ps
                        for o in range(KTG):
                            kt = g * KTG + o
                            nc.tensor.matmul(
                                ps[:, o, :],
                                lhsT=kT[hh : hh + HD, h // HP,
                                        kt * P : (kt + 1) * P],
                                rhs=q_h,
                                start=True,
                                stop=True,
                            )

                    # software-pipelined: scores(g+1) is issued to the PE
                    # ahead of AV(g), so the PE never sits behind an AV that
                    # is itself waiting on exp(g).
                    scores(0)
                    for g in range(NGx):
                        if g + 1 < NGx:
                            scores(g + 1)
                        ps = pstiles.pop(g)
                        ex = exp_.tile([P, KTG, W], bf16, tag="ex")
                        nc.scalar.activation(
                            out=ex[:, :, :], in_=ps[:, :, :], func=AF.Exp,
                            scale=1.0 / float(np.sqrt(HD)),
                        )
                        if mfull is not None:
                            nc.vector.tensor_mul(
                                ex[:, :, :], ex[:, :, :],
                                mfull[:, g * KTG : (g + 1) * KTG, :],
                            )
                        elif msb is not None:
                            # overlap of this group's slots with the diagonal
                            # region [NTx-QT, NTx)
                            lo = max(g * KTG, NTx - QT)
                            hi = (g + 1) * KTG
                            if lo < hi:
                                nc.vector.tensor_mul(
                                    ex[:, lo - g * KTG : KTG, :],
                                    ex[:, lo - g * KTG : KTG, :],
                                    msb[:, lo - (NTx - QT) : hi - (NTx - QT), :],
                                )
                        for o in range(KTG):
                            kt = g * KTG + o
                            nc.tensor.matmul(
                                po[0:HD1, :],
                                lhsT=v[:, kt, h, :],
                                rhs=ex[:, o, :],
                                start=(g == 0 and o == 0),
                                stop=(g == NGx - 1 and o == KTG - 1),
                            )
                    dinv = dnp.tile([1, W], f32, tag="dinv")
                    nc.vector.reciprocal(dinv[0:1, :], po[HD:HD1, :])
                    dinvb = dnp.tile([HD, W], f32, tag="dinvb")
                    nc.gpsimd.partition_broadcast(
                        dinvb[0:HD, :], dinv[0:1, :], channels=HD
                    )
                    nc.vector.tensor_mul(
                        aoT[hh : hh + HD, h // HP, :],
                        po[0:HD, :],
                        dinvb[0:HD, :],
                    )

        # =========== layernorm (transposed layout, f32 internals) ===========
        def ln_t(pre, out_bf, out_f, g_sb, b_sb, lpp, lp):
            """pre: [P, DT, W] f32 sbuf; out_bf bf16 (or None), out_f f32
            (or None; at least one)."""
            acc = lp.tile([P, W], f32, tag="lnacc")
            nc.vector.tensor_add(acc[:, :], pre[:, 0, :], pre[:, 1, :])
            for d in range(2, DT):
                nc.vector.tensor_add(acc[:, :], acc[:, :], pre[:, d, :])
            sqa = lp.tile([P, W], f32, tag="lnsqa")
            nc.scalar.square(sqa[:, :], pre[:, 0, :])
            for d in range(1, DT):
                sqt = lp.tile([P, W], f32, tag="lnsqt")
                nc.scalar.square(sqt[:, :], pre[:, d, :])
                nc.vector.tensor_add(sqa[:, :], sqa[:, :], sqt[:, :])
            sums = lpp.tile([1, W], f32, tag="lnsums")
            nc.tensor.matmul(sums[0:1, :], lhsT=ones_p1[:, :],
                             rhs=acc[:, :], start=True, stop=True)
            sqs = lpp.tile([1, W], f32, tag="lnsqs")
            nc.tensor.matmul(sqs[0:1, :], lhsT=ones_p1[:, :],
                             rhs=sqa[:, :], start=True, stop=True)
            mu = lp.tile([1, W], f32, tag="lnmu")
            nc.vector.tensor_scalar_mul(mu[0:1, :], sums[0:1, :], 1.0 / D)
            ex2 = lp.tile([1, W], f32, tag="lnex2")
            nc.vector.tensor_scalar_mul(ex2[0:1, :], sqs[0:1, :], 1.0 / D)
            mu2 = lp.tile([1, W], f32, tag="lnmu2")
            nc.scalar.square(mu2[0:1, :], mu[0:1, :])
            var = lp.tile([1, W], f32, tag="lnvar")
            nc.vector.tensor_sub(var[0:1, :], ex2[0:1, :], mu2[0:1, :])
            sd = lp.tile([1, W], f32, tag="lnsd")
            nc.scalar.activation(out=sd[0:1, :], in_=var[0:1, :], func=AF.Sqrt,
                                 bias=eps_t[0:1, :], scale=1.0)
            rstd = lp.tile([1, W], f32, tag="lnrstd")
            nc.vector.reciprocal(rstd[0:1, :], sd[0:1, :])
            mub = lpp.tile([P, W], f32, tag="lnmub")
            nc.tensor.matmul(mub[:, :], lhsT=ones_1p[0:1, :],
                             rhs=mu[0:1, :], start=True, stop=True)
            rstdb = lpp.tile([P, W], f32, tag="lnrstdb")
            nc.tensor.matmul(rstdb[:, :], lhsT=ones_1p[0:1, :],
                             rhs=rstd[0:1, :], start=True, stop=True)
            mubs = lp.tile([P, W], f32, tag="lnmubs")
            nc.vector.tensor_copy(mubs[:, :], mub[:, :])
            rstdbs = lp.tile([P, W], f32, tag="lnrstdbs")
            nc.vector.tensor_copy(rstdbs[:, :], rstdb[:, :])
            for d in range(DT):
                t1 = lp.tile([P, W], f32, tag="lnt1")
                nc.vector.tensor_sub(t1[:, :], pre[:, d, :], mubs[:, :])
                of = out_f[:, d, :] if out_f is not None else None
                if of is not None:
                    nc.vector.tensor_mul(of, t1[:, :], rstdbs[:, :])
                    if g_sb is not None:
                        nc.vector.tensor_scalar_mul(of, of, g_sb[:, d : d + 1])
                    if b_sb is not None:
                        nc.vector.tensor_scalar_add(of, of, b_sb[:, d : d + 1])
                    if out_bf is not None:
                        nc.vector.tensor_copy(out_bf[:, d, :], of)
                else:
                    ob = out_bf[:, d, :]
                    nc.vector.tensor_mul(ob, t1[:, :], rstdbs[:, :])
                    if g_sb is not None:
                        nc.vector.tensor_scalar_mul(ob, ob, g_sb[:, d : d + 1])
                    if b_sb is not None:
                        nc.vector.tensor_scalar_add(ob, ob, b_sb[:, d : d + 1])

        # =========== out-projection + residual + LN ===========
        def proj_resid_ln(owT, obsb, residT, g_sb, b_sb, out_bf, out_f):
            with tc.tile_pool(name="pr_w", bufs=2) as wp, \
                 tc.tile_pool(name="pr_t", bufs=2) as lp, \
                 tc.tile_pool(name="pr_pre", bufs=1) as prep, \
                 tc.tile_pool(name="pr_ps", bufs=2, space="PSUM") as psp, \
                 tc.tile_pool(name="pr_lnps", bufs=1, space="PSUM") as lpp:
                pre = prep.tile([P, DT, W], f32, tag="pre")
                for dg in range(DT // G4):
                    wsl = wp.tile([P, DT, G4 * P], bf16, tag="prw")
                    nc.sync.dma_start(
                        out=wsl[:, :, :],
                        in_=owT[:, dg * G4 * P : (dg + 1) * G4 * P]
                        .rearrange("(t p) v -> p t v", p=P),
                    )
                    for j in range(G4):
                        d = dg * G4 + j
                        ps = psp.tile([P, W], f32, tag="prps")
                        for dt in range(DT):
                            nc.tensor.matmul(
                                ps[:, :], lhsT=wsl[:, dt, j * P : (j + 1) * P],
                                rhs=aoT[:, dt, :],
                                start=(dt == 0), stop=(dt == DT - 1),
                            )
                        if obsb is not None:
                            tmp = lp.tile([P, W], f32, tag="prtmp")
                            nc.scalar.activation(out=tmp[:, :], in_=ps[:, :],
                                                 func=AF.Identity,
                                                 bias=obsb[:, d : d + 1], scale=1.0)
                            nc.vector.tensor_add(pre[:, d, :], tmp[:, :],
                                                 residT[:, d, :])
                        else:
                            nc.vector.tensor_add(pre[:, d, :], ps[:, :],
                                                 residT[:, d, :])
                ln_t(pre, out_bf, out_f, g_sb, b_sb, lpp, lp)

        # ================= block 1: self-attention =================
        with tc.tile_pool(name="kv1", bufs=1) as kvp1:
            kT1 = kvp1.tile([P, DT, S], bf16)
            v1 = kvp1.tile([P, NT, H, HD1], bf16)
            with tc.tile_pool(name="xs1", bufs=1) as xsp1:
                xs = xsp1.tile([P, DT, S], bf16)
                nc.sync.dma_start(out=xs[:, :, :],
                                  in_=xTr.rearrange("(t p) s -> p t s", p=P))
                # the host always rotates key order so this core's quarter
                # sits in the last QT slots (uniform across cores); any mask
                # data is supplied in rotated coordinates.
                xq_off = S - W
                xloc = xs[:, :, xq_off : xq_off + W]
                nc.vector.tensor_copy(xlocT[:, :, :], xloc)
                qkv_phase(xs, xloc, qkvwT1, kT1, v1, qkb1sb, vb1sb, zv1sb)
            attn_phase(kT1, v1, m1sb, m1fsb)
        proj_resid_ln(owT1, ob1sb, xlocT, lns["g1"], lns["b1"], x1T, xf)

        # ---- local K2/V2/Q2 from x1, then all-gather K2|V2 in group ----
        assert cfg.use_collective
        with tc.tile_pool(name="kvloc", bufs=1) as kvlp:
            klocT = kvlp.tile([P, DT, W], bf16)
            vloc = kvlp.tile([P, QT, H, HD1], bf16)
            qkv_phase(x1T, x1T, qkvwT2, klocT, vloc, qkb2sb, vb2sb, zv2sb)
            nc.sync.dma_start(
                out=kvs[0 : D * W].rearrange("(t p w) -> p t w", p=P, w=W),
                in_=klocT[:, :, :],
            )
            nc.sync.dma_start(
                out=kvs[D * W : LKV].rearrange("(q p h d) -> p q h d",
                                               p=P, h=H, d=HD1),
                in_=vloc[:, :, :, :],
            )
        if cfg.fake_gather:
            for g in range(4):
                nc.sync.dma_start(out=agkv[g * LKV : (g + 1) * LKV],
                                  in_=kvs[:])
        else:
            nc.gpsimd.collective_compute(
                "AllGather",
                bass.mybir.AluOpType.bypass,
                replica_groups=[[0, 1, 2, 3], [4, 5, 6, 7]],
                ins=[kvs[:]],
                outs=[agkv[:]],
            )

        # ================= block 2: cross-attention =================
        with tc.tile_pool(name="kv2", bufs=1) as kvp2:
            kT2 = kvp2.tile([P, DT, S], bf16)
            v2 = kvp2.tile([P, NT, H, HD1], bf16)
            for g in range(4):
                nc.sync.dma_start(
                    out=kT2[:, :, g * W : (g + 1) * W],
                    in_=agkv[g * LKV : g * LKV + D * W]
                    .rearrange("(t p w) -> p t w", p=P, w=W),
                )
                nc.sync.dma_start(
                    out=v2[:, g * QT : (g + 1) * QT, :, :],
                    in_=agkv[g * LKV + D * W : (g + 1) * LKV]
                    .rearrange("(q p h d) -> p q h d", p=P, h=H, d=HD1),
                )
            attn_phase(kT2, v2, None, None)
        proj_resid_ln(owT2, ob2sb, xf, lns["g2"], lns["b2"], x2T, xf)

        # ================= FFN =================
        with tc.tile_pool(name="ffh", bufs=1) as fhp, \
             tc.tile_pool(name="ffw", bufs=2) as wp, \
             tc.tile_pool(name="fft", bufs=1) as lp, \
             tc.tile_pool(name="ffpre", bufs=1) as prep:
            hT = fhp.tile([P, FT, W], bf16)
            with tc.tile_pool(name="ffps1", bufs=2, space="PSUM") as psp:
                for fg in range(FT // G4):
                    wsl = wp.tile([P, DT, G4 * P], bf16, tag="f1w")
                    nc.sync.dma_start(
                        out=wsl[:, :, :],
                        in_=w1T[:, fg * G4 * P : (fg + 1) * G4 * P]
                        .rearrange("(t p) v -> p t v", p=P),
                    )
                    for j in range(G4):
                        f = fg * G4 + j
                        ps = psp.tile([P, W], f32, tag="f1ps")
                        for dt in range(DT):
                            nc.tensor.matmul(
                                ps[:, :], lhsT=wsl[:, dt, j * P : (j + 1) * P],
                                rhs=x2T[:, dt, :],
                                start=(dt == 0), stop=(dt == DT - 1),
                            )
                        if fb1sb is not None:
                            nc.scalar.activation(out=hT[:, f, :], in_=ps[:, :],
                                                 func=AF.Relu,
                                                 bias=fb1sb[:, f : f + 1], scale=1.0)
                        else:
                            nc.scalar.activation(out=hT[:, f, :], in_=ps[:, :],
                                                 func=AF.Relu)
            pre = prep.tile([P, DT, W], f32, tag="ffpre")
            FTC = min(8, FT)  # w2 staging chunk (ft tiles per DMA)
            with tc.tile_pool(name="ffps2", bufs=1, space="PSUM") as psq, \
                 tc.tile_pool(name="fflnps", bufs=1, space="PSUM") as lpp:
                for dg in range(DT // G4):
                    ps4 = []
                    for j in range(G4):
                        ps4j = psq.tile([P, W], f32, tag="f2ps%d" % j)
                        ps4.append(ps4j)
                    for fc in range(FT // FTC):
                        w2sl = wp.tile([P, FTC, G4 * P], bf16, tag="f2w")
                        nc.sync.dma_start(
                            out=w2sl[:, :, :],
                            in_=w2T[fc * FTC * P : (fc + 1) * FTC * P,
                                    dg * G4 * P : (dg + 1) * G4 * P]
                            .rearrange("(t p) v -> p t v", p=P),
                        )
                        for fo in range(FTC):
                            ft = fc * FTC + fo
                            for j in range(G4):
                                nc.tensor.matmul(
                                    ps4[j][:, :],
                                    lhsT=w2sl[:, fo, j * P : (j + 1) * P],
                                    rhs=hT[:, ft, :],
                                    start=(ft == 0), stop=(ft == FT - 1),
                                )
                    for j in range(G4):
                        d = dg * G4 + j
                        if fb2sb is not None:
                            tmp = lp.tile([P, W], f32, tag="f2tmp")
                            nc.scalar.activation(out=tmp[:, :], in_=ps4[j][:, :],
                                                 func=AF.Identity,
                                                 bias=fb2sb[:, d : d + 1], scale=1.0)
                            nc.vector.tensor_add(pre[:, d, :], tmp[:, :],
                                                 xf[:, d, :])
                        else:
                            nc.vector.tensor_add(pre[:, d, :], ps4[j][:, :],
                                                 xf[:, d, :])
                outp = prep.tile([P, DT, W], f32, tag="ffout")
                ln_t(pre, None, outp, lns["g3"], lns["b3"], lpp, lp)
                nc.sync.dma_start(
                    out=out.rearrange("(t p) w -> p t w", p=P),
                    in_=outp[:, :, :])


def make_program(cfg, fl):
    from concourse import bacc
    import concourse.tile as tile

    nc = bacc.Bacc("TRN2", target_bir_lowering=False, debug=False,
                   num_devices=8)
    with tile.TileContext(nc) as tc:
        with nc.allow_low_precision(reason="bf16 kernel, rel-err gate 2e-2"):
            _build(nc, tc, cfg, fl)
    nc.compile()
    return nc


def prep_inputs(inputs, cfg):
    """Host-side data prep. Returns (in_maps, fl)."""
    import ml_dtypes

    bf = ml_dtypes.bfloat16
    B, S, D, H, DFF, W, NT, QT = (cfg.B, cfg.S, cfg.D, cfg.H, cfg.DFF,
                                  cfg.W, cfg.NT, cfg.QT)
    f = np.float32
    x = np.asarray(inputs["x"], f)
    enc = np.asarray(inputs["enc_out"])
    trg = np.asarray(inputs["trg_mask"])
    fl = Flags()
    fl.qkb1 = bool(np.any(inputs["qkv_b1"]))
    fl.qkb2 = bool(np.any(inputs["qkv_b2"]))
    fl.vb1 = bool(np.any(np.asarray(inputs["qkv_b1"])[2 * D :]))
    fl.vb2 = bool(np.any(np.asarray(inputs["qkv_b2"])[2 * D :]))
    fl.ob1 = bool(np.any(inputs["out_b1"]))
    fl.ob2 = bool(np.any(inputs["out_b2"]))
    fl.fb1 = bool(np.any(inputs["ff_b1"]))
    fl.fb2 = bool(np.any(inputs["ff_b2"]))
    fl.g1 = not bool(np.all(np.asarray(inputs["ln1_g"]) == 1))
    fl.b1 = bool(np.any(inputs["ln1_b"]))
    fl.g2 = not bool(np.all(np.asarray(inputs["ln2_g"]) == 1))
    fl.b2 = bool(np.any(inputs["ln2_b"]))
    fl.g3 = not bool(np.all(np.asarray(inputs["ln3_g"]) == 1))
    fl.b3 = bool(np.any(inputs["ln3_b"]))
    fl.m1 = not bool(np.all(trg != 0))
    tril = np.tril(np.ones((S, S), np.int32))
    is_tril = (trg.shape[0] == 1 and
               bool(np.array_equal((trg[0, 0] != 0).astype(np.int32), tril)))
    fl.m1full = fl.m1 and not is_tril
    fl.zv2 = bool(np.any(enc == 0))

    def bcast(a):
        return np.ascontiguousarray(np.asarray(a, f).T.astype(bf))

    shared = {
        "qkvwT1": bcast(inputs["qkv_w1"]),
        "qkvwT2": bcast(inputs["qkv_w2"]),
        "owT1": bcast(inputs["out_w1"]),
        "owT2": bcast(inputs["out_w2"]),
        "w1T": bcast(inputs["ff_w1"]),
        "w2T": bcast(inputs["ff_w2"]),
    }
    if fl.qkb1:
        shared["qkvb1"] = np.asarray(inputs["qkv_b1"], f)
    if fl.qkb2:
        shared["qkvb2"] = np.asarray(inputs["qkv_b2"], f)
    if fl.vb1:
        shared["vb1"] = np.broadcast_to(
            np.asarray(inputs["qkv_b1"], f)[2 * D :], (P, D)).astype(bf)
    if fl.vb2:
        shared["vb2"] = np.broadcast_to(
            np.asarray(inputs["qkv_b2"], f)[2 * D :], (P, D)).astype(bf)
    if fl.ob1:
        shared["ob1"] = np.asarray(inputs["out_b1"], f)
    if fl.ob2:
        shared["ob2"] = np.asarray(inputs["out_b2"], f)
    if fl.fb1:
        shared["fb1"] = np.asarray(inputs["ff_b1"], f)
    if fl.fb2:
        shared["fb2"] = np.asarray(inputs["ff_b2"], f)
    for nm, key, use in [("g1", "ln1_g", fl.g1), ("b1", "ln1_b", fl.b1),
                         ("g2", "ln2_g", fl.g2), ("b2", "ln2_b", fl.b2),
                         ("g3", "ln3_g", fl.g3), ("b3", "ln3_b", fl.b3)]:
        if use:
            shared[nm] = np.asarray(inputs[key], f)
    if fl.m1 and not fl.m1full:
        # constant triangular mask for the last QT slots, same on all cores:
        # m1c[k', s, q'] = 1 if s*P + k' <= q'
        kk = np.arange(P)[:, None, None]
        ss = np.arange(QT)[None, :, None]
        qq = np.arange(W)[None, None, :]
        shared["m1c"] = ((ss * P + kk) <= qq).astype(bf)

    xTb = [np.ascontiguousarray(x[b].T).astype(bf) for b in range(B)]
    in_maps = []
    for c in range(8):
        b, r = c // 4, c % 4
        m = dict(shared)
        # rotate key tiles: slot t holds physical tile p(t) = (t+(r+1)*QT)%NT
        perm = [(t + (r + 1) * QT) % NT for t in range(NT)]
        xt = xTb[b].reshape(D, NT, P)
        m["xTr"] = np.ascontiguousarray(
            xt[:, perm, :].reshape(D, S))
        if fl.m1 and not fl.m1full:
            zv = np.zeros((P, NT), f)
            for t in range(NT):
                if perm[t] < (r + 1) * QT:
                    zv[:, t] = 1
            m["zv1"] = zv
        if fl.m1full:
            # generic multiplicative mask in rotated key coordinates
            tb = trg[b] if trg.shape[0] == B else trg[0]
            blk = (tb[0, r * W : (r + 1) * W, :] != 0).astype(f)  # [W, S](q,k)
            mk = blk.T.reshape(NT, P, W)  # [kt, k', q]
            m["m1f"] = np.ascontiguousarray(mk[perm]).astype(bf)
        if fl.zv2:
            # this core's own quarter only (keys it contributes to the
            # gathered V2) -- applied before the gather, so consumers see
            # already-zeroed rows
            eb = (np.asarray(enc)[b, 0, 0, r * W : (r + 1) * W] != 0).astype(f)
            m["zv2"] = np.ascontiguousarray(
                eb.reshape(W // P, P).T).astype(f)
        in_maps.append(m)
    return in_maps, fl


def kernel_with_results(**inputs):
    from concourse.bass_utils import run_bass_kernel_spmd

    cfg = Cfg()
    x = np.asarray(inputs["x"])
    assert x.shape == (cfg.B, cfg.S, cfg.D), x.shape
    in_maps, fl = prep_inputs(inputs, cfg)
    nc = make_program(cfg, fl)
    res = run_bass_kernel_spmd(nc, in_maps, list(range(8)))
    y = np.empty((cfg.B, cfg.S, cfg.D), np.float32)
    for c in range(8):
        b, r = c // 4, c % 4
        y[b, r * cfg.W : (r + 1) * cfg.W, :] = res.results[c]["out"].T
    return y, res


def kernel(**inputs):
    return kernel_with_results(**inputs)[0]


# revision 31
# speedup vs baseline: 1.3152x; 1.0014x over previous
"""Trainium2 Bass kernel for nn_DecoderLayer (dense transformer decoder layer).

Strategy (8 NeuronCores, full inputs in / full output out):
  - core c handles batch b = c//4 and query-quarter r = c%4 (rows [r*S/4, (r+1)*S/4)).
  - All matmul operands are bf16 (PSUM accumulation fp32); activations are
    kept TRANSPOSED on-chip (x^T [D, n]) so projections run with the
    contraction dim on partitions.
  - K^T / V / Q^T live entirely in SBUF (no HBM round-trip).
  - Self-attention causal masking: the host rotates each core's key order so
    the 4 "diagonal" key tiles sit at fixed slots (last group); keys fully
    below the diagonal get their V rows zeroed at projection time (per-key
    0/1 scale fused into the PSUM evacuation), so only the last exp group
    needs a (core-independent, constant) triangular multiplicative mask.
  - Cross-attention key masking (enc mask) uses the same V-row zeroing.
  - Softmax denominators come free from a ones column appended to V.
  - The single collective: AllGather of x1 (post-LN1) within each 4-core
    batch group, needed because cross-attention K2/V2 are projections of the
    full x1.
  - LayerNorm runs in transposed layout: cross-partition sums via ones-matmul
    on the PE, stats broadcast back to [128, W] via ones-matmul.
"""

import sys

if "/opt/trn_rl_repo" not in sys.path:
    sys.path.insert(0, "/opt/trn_rl_repo")

import numpy as np

P = 128
HD = 64
HD1 = HD + 1
EPS = 1e-5


class Cfg:
    def __init__(self, B=2, S=2048, D=1024, H=16, DFF=4096, use_collective=True,
                 fake_gather=False):
        self.B, self.S, self.D, self.H, self.DFF = B, S, D, H, DFF
        self.fake_gather = fake_gather
        self.W = S // 4            # local query rows per core
        self.DT = D // P           # feature-dim tiles
        self.NT = S // P           # sequence tiles (keys)
        self.FT = DFF // P         # ffn hidden tiles
        self.HP = P // HD          # heads per partition-tile (2)
        self.QT = self.W // P      # key tiles per query quarter (diag tiles)
        self.KTG = min(2, self.NT)    # k-tiles per exp group
        self.NG = self.NT // self.KTG
        self.VCW = min(512, D)        # v-dout chunk width
        self.VCN = D // self.VCW
        self.HPC = self.VCW // HD     # heads per v-chunk
        self.KCW = min(512, self.S)   # k-proj token chunk width
        self.KCN = self.S // self.KCW
        self.G4 = min(4, self.DT)
        self.use_collective = use_collective
        assert D == H * HD
        assert self.W % P == 0 and D % P == 0 and DFF % P == 0 and S % P == 0


class Flags:
    def __init__(self):
        self.qkb1 = self.vb1 = self.ob1 = False
        self.qkb2 = self.vb2 = self.ob2 = False
        self.fb1 = self.fb2 = False
        self.g1 = self.b1 = self.g2 = self.b2 = self.g3 = self.b3 = False
        self.m1 = True      # trg mask active (tril fast path)
        self.m1full = False  # generic (non-tril) trg mask: full mult tiles
        self.zv2 = False    # enc mask active -> zero V2 rows


def _build(nc, tc, cfg, fl):
    import concourse.bass as bass
    import concourse.mybir as mybir
    import concourse.tile as tile  # noqa: F401
    from contextlib import ExitStack

    AF = mybir.ActivationFunctionType
    f32 = mybir.dt.float32
    bf16 = mybir.dt.bfloat16

    B, S, D, H, DFF = cfg.B, cfg.S, cfg.D, cfg.H, cfg.DFF
    W, DT, NT, FT, HP = cfg.W, cfg.DT, cfg.NT, cfg.FT, cfg.HP
    QT, KTG, NG = cfg.QT, cfg.KTG, cfg.NG
    VCW, VCN, HPC = cfg.VCW, cfg.VCN, cfg.HPC
    KCW, KCN, G4 = cfg.KCW, cfg.KCN, cfg.G4

    # ---------------- DRAM parameters ----------------
    def din(name, shape, dt=bf16):
        return nc.dram_tensor(name, shape, dt, kind="ExternalInput").ap()

    xTr = din("xTr", [D, S])          # rotated x^T for this core
    qkvwT1 = din("qkvwT1", [D, 3 * D])
    qkvwT2 = din("qkvwT2", [D, 3 * D])
    owT1 = din("owT1", [D, D])
    owT2 = din("owT2", [D, D])
    w1T = din("w1T", [D, DFF])
    w2T = din("w2T", [DFF, D])
    m1c = din("m1c", [P, QT, W]) if (fl.m1 and not fl.m1full) else None
    m1f = din("m1f", [NT, P, W]) if fl.m1full else None
    zv1 = din("zv1", [P, NT], f32) if fl.m1 and not fl.m1full else None
    zv2d = din("zv2", [P, QT], f32) if fl.zv2 else None
    qkvb1 = din("qkvb1", [3 * D], f32) if fl.qkb1 else None
    qkvb2 = din("qkvb2", [3 * D], f32) if fl.qkb2 else None
    vb1 = din("vb1", [P, D]) if fl.vb1 else None
    vb2 = din("vb2", [P, D]) if fl.vb2 else None
    ob1 = din("ob1", [D], f32) if fl.ob1 else None
    ob2 = din("ob2", [D], f32) if fl.ob2 else None
    fb1d = din("fb1", [DFF], f32) if fl.fb1 else None
    fb2d = din("fb2", [D], f32) if fl.fb2 else None
    lnp = {}
    for nm, use in [("g1", fl.g1), ("b1", fl.b1), ("g2", fl.g2),
                    ("b2", fl.b2), ("g3", fl.g3), ("b3", fl.b3)]:
        lnp[nm] = din(nm, [D], f32) if use else None
    out = nc.dram_tensor("out", [D, W], f32, kind="ExternalOutput").ap()

    es = ExitStack()
    with es:
        dramp = es.enter_context(tc.tile_pool(name="dram", bufs=1, space="DRAM"))
        LKV = D * W + W * H * HD1  # flat K2loc + V2loc staging elements
        if cfg.use_collective:
            kvs = dramp.tile([LKV], bf16)
            agkv = dramp.tile([4 * LKV], bf16)

        const = es.enter_context(tc.tile_pool(name="const", bufs=1))
        ones_p1 = const.tile([P, 1], f32)
        nc.vector.memset(ones_p1[:, :], 1.0)
        ones_1p = const.tile([1, P], f32)
        nc.vector.memset(ones_1p[0:1, :], 1.0)
        ones_col = const.tile([P, HPC, 1], bf16)
        nc.vector.memset(ones_col[:, :, :], 1.0)
        eps_t = const.tile([1, 1], f32)
        nc.vector.memset(eps_t[0:1, :], EPS)

        def ldvec(dram_vec, n_tiles, name):
            """[D]-style f32 vector -> [P, n_tiles] sbuf tile."""
            t = const.tile([P, n_tiles], f32, tag=name)
            nc.sync.dma_start(
                out=t[:, :],
                in_=dram_vec.rearrange("(t p) -> p t", p=P),
            )
            return t

        qkb1sb = ldvec(qkvb1[0 : 2 * D], 2 * DT, "qkb1") if fl.qkb1 else None
        qkb2sb = ldvec(qkvb2[0 : 2 * D], 2 * DT, "qkb2") if fl.qkb2 else None
        ob1sb = ldvec(ob1, DT, "ob1") if fl.ob1 else None
        ob2sb = ldvec(ob2, DT, "ob2") if fl.ob2 else None
        fb1sb = ldvec(fb1d, FT, "fb1") if fl.fb1 else None
        fb2sb = ldvec(fb2d, DT, "fb2") if fl.fb2 else None
        lns = {k: (ldvec(v, DT, "ln" + k) if v is not None else None)
               for k, v in lnp.items()}
        vb1sb = None
        if fl.vb1:
            vb1sb = const.tile([P, D], bf16, tag="vb1")
            nc.sync.dma_start(out=vb1sb[:, :], in_=vb1[:, :])
        vb2sb = None
        if fl.vb2:
            vb2sb = const.tile([P, D], bf16, tag="vb2")
            nc.sync.dma_start(out=vb2sb[:, :], in_=vb2[:, :])
        zv1sb = None
        if zv1 is not None:
            zv1sb = const.tile([P, NT], f32, tag="zv1")
            nc.sync.dma_start(out=zv1sb[:, :], in_=zv1[:, :])
        zv2sb = None
        if zv2d is not None:
            zv2sb = const.tile([P, QT], f32, tag="zv2")
            nc.sync.dma_start(out=zv2sb[:, :], in_=zv2d[:, :])
        m1sb = None
        if m1c is not None:
            m1sb = const.tile([P, QT, W], bf16, tag="m1c")
            nc.sync.dma_start(out=m1sb[:, :, :], in_=m1c[:, :, :])
        m1fsb = None
        if m1f is not None:
            m1fsb = const.tile([P, NT, W], bf16, tag="m1f")
            nc.sync.dma_start(out=m1fsb[:, :, :], in_=m1f.rearrange("n p w -> p n w"))

        # persistent mid tensors
        midp = es.enter_context(tc.tile_pool(name="mid", bufs=1))
        qT = midp.tile([P, DT, W], bf16)      # Q^T local (reused block2)
        aoT = midp.tile([P, DT, W], bf16)     # attention out^T (reused)
        x1T = midp.tile([P, DT, W], bf16)     # x1 local
        x2T = midp.tile([P, DT, W], bf16)     # x2 local
        xlocT = midp.tile([P, DT, W], bf16)   # this core's x quarter (resid 1)
        xf = midp.tile([P, DT, W], f32)       # f32 residual backbone (x1, x2)

        # =========== QKV projection (into SBUF K/V/Q) ===========
        def qkv_phase(xsb, xq, wT, kT, v, qkb, vbsb, zvsb):
            """xsb: [P, DT, Sx] bf16 x^T source (tokens = key order);
            xq: [P, DT, W] bf16 x^T source for this core's queries;
            writes kT [P, DT, Sx] sbuf, v [P, NT_x, H, HD1] sbuf, qT."""
            Sx = xsb.shape[2]
            NTx = Sx // P
            KCWx = min(KCW, Sx)
            KCNx = Sx // KCWx
            with tc.tile_pool(name="qkv_w", bufs=2) as wp, \
                 tc.tile_pool(name="qkv_ps", bufs=2, space="PSUM") as psp:
                # ---- K^T ----
                wall = wp.tile([P, DT, D], bf16, tag="wall")
                nc.sync.dma_start(
                    out=wall[:, :, :],
                    in_=wT[:, D : 2 * D].rearrange("(t p) v -> p t v", p=P),
                )
                for nch in range(KCNx):
                    for dk in range(DT):
                        ps = psp.tile([P, KCWx], f32, tag="kps")
                        for dt in range(DT):
                            nc.tensor.matmul(
                                ps[:, :],
                                lhsT=wall[:, dt, dk * P : (dk + 1) * P],
                                rhs=xsb[:, dt, nch * KCWx : (nch + 1) * KCWx],
                                start=(dt == 0),
                                stop=(dt == DT - 1),
                            )
                        if qkb is not None:
                            nc.scalar.activation(
                                out=kT[:, dk, nch * KCWx : (nch + 1) * KCWx],
                                in_=ps[:, :], func=AF.Identity,
                                bias=qkb[:, DT + dk : DT + dk + 1], scale=1.0,
                            )
                        else:
                            nc.scalar.activation(
                                out=kT[:, dk, nch * KCWx : (nch + 1) * KCWx],
                                in_=ps[:, :], func=AF.Copy,
                            )
                # ---- Q^T local [D, W] ----
                wall = wp.tile([P, DT, D], bf16, tag="wall")
                nc.sync.dma_start(
                    out=wall[:, :, :],
                    in_=wT[:, 0:D].rearrange("(t p) v -> p t v", p=P),
                )
                for dq in range(DT):
                    ps = psp.tile([P, W], f32, tag="qps")
                    for dt in range(DT):
                        nc.tensor.matmul(
                            ps[:, :],
                            lhsT=wall[:, dt, dq * P : (dq + 1) * P],
                            rhs=xq[:, dt, :],
                            start=(dt == 0),
                            stop=(dt == DT - 1),
                        )
                    if qkb is not None:
                        nc.scalar.activation(
                            out=qT[:, dq, :], in_=ps[:, :], func=AF.Identity,
                            bias=qkb[:, dq : dq + 1], scale=1.0,
                        )
                    else:
                        nc.scalar.activation(
                            out=qT[:, dq, :], in_=ps[:, :], func=AF.Copy,
                        )
                # ---- V natural [n, dout] + ones column ----
                wall = wp.tile([P, DT, D], bf16, tag="wall")
                nc.sync.dma_start(
                    out=wall[:, :, :],
                    in_=wT[:, 2 * D : 3 * D].rearrange("(t p) v -> p t v", p=P),
                )
                for nt in range(NTx):
                    for vc in range(VCN):
                        ps = psp.tile([P, VCW], f32, tag="vps")
                        for dt in range(DT):
                            nc.tensor.matmul(
                                ps[:, :],
                                lhsT=xsb[:, dt, nt * P : (nt + 1) * P],
                                rhs=wall[:, dt, vc * VCW : (vc + 1) * VCW],
                                start=(dt == 0),
                                stop=(dt == DT - 1),
                            )
                        dst = v[:, nt, vc * HPC : (vc + 1) * HPC, 0:HD]
                        psv = ps.rearrange("p (h d) -> p h d", d=HD)
                        if vbsb is not None:
                            # bias first, then per-key zeroing (mask applies
                            # to the biased value)
                            nc.scalar.activation(out=dst, in_=psv, func=AF.Copy)
                            nc.vector.tensor_add(
                                dst, dst,
                                vbsb[:, vc * VCW : (vc + 1) * VCW].rearrange(
                                    "p (h d) -> p h d", d=HD),
                            )
                            if zvsb is not None:
                                nc.vector.tensor_scalar_mul(
                                    dst, dst, zvsb[:, nt : nt + 1])
                        elif zvsb is not None:
                            nc.scalar.activation(
                                out=dst, in_=psv, func=AF.Copy,
                                scale=zvsb[:, nt : nt + 1],
                            )
                        else:
                            nc.scalar.activation(out=dst, in_=psv, func=AF.Copy)
                        oc = v[:, nt, vc * HPC : (vc + 1) * HPC, HD:HD1]
                        if zvsb is not None:
                            nc.scalar.activation(
                                out=oc, in_=ones_col[:, :, :], func=AF.Copy,
                                scale=zvsb[:, nt : nt + 1],
                            )
                        else:
                            nc.vector.memset(oc, 1.0)

        # =========== attention phase ===========
        def attn_phase(kT, v, msb, mfull):
            """msb: [P, QT, W] triangular mask on the LAST QT slots (or None);
            mfull: [P, NT, W] generic multiplicative mask (or None).
            Score PSUM is manually double-buffered (two KTG-slot halves of one
            [P, 2*KTG, W] tile) so the next group's matmuls never wait on the
            previous group's exp."""
            NTx = v.shape[1]
            NGx = NTx // KTG
            with tc.tile_pool(name="at_ex", bufs=3) as exp_, \
                 tc.tile_pool(name="at_dn", bufs=2) as dnp, \
                 tc.tile_pool(name="at_ps", bufs=2, space="PSUM") as psp, \
                 tc.tile_pool(name="at_po", bufs=2, space="PSUM") as pop:
                for h in range(H):
                    hh = (h % HP) * HD
                    q_h = qT[hh : hh + HD, h // HP, :]
                    po = pop.tile([P, W], f32, tag="po")
                    pstiles = {}

                    def scores(g):
                        ps = psp.tile([P, KTG, W], f32, tag="sc")
                        pstiles[g] = ps
                        for o in range(KTG):
                            kt = g * KTG + o
                            nc.tensor.matmul(
                                ps[:, o, :],
                                lhsT=kT[hh : hh + HD, h // HP,
                                        kt * P : (kt + 1) * P],
                                rhs=q_h,
                                start=True,
                                stop=True,
                            )

                    # software-pipelined: scores(g+1) is issued to the PE
                    # ahead of AV(g), so the PE never sits behind an AV that
                    # is itself waiting on exp(g).
                    scores(0)
                    for g in range(NGx):
                        if g + 1 < NGx:
                            scores(g + 1)
                        ps = pstiles.pop(g)
                        ex = exp_.tile([P, KTG, W], bf16, tag="ex")
                        nc.scalar.activation(
                            out=ex[:, :, :], in_=ps[:, :, :], func=AF.Exp,
                            scale=1.0 / float(np.sqrt(HD)),
                        )
                        if mfull is not None:
                            nc.vector.tensor_mul(
                                ex[:, :, :], ex[:, :, :],
                                mfull[:, g * KTG : (g + 1) * KTG, :],
                            )
                        elif msb is not None:
                            # overlap of this group's slots with the diagonal
                            # region [NTx-QT, NTx)
                            lo = max(g * KTG, NTx - QT)
                            hi = (g + 1) * KTG
                            if lo < hi:
                                nc.vector.tensor_mul(
                                    ex[:, lo - g * KTG : KTG, :],
                                    ex[:, lo - g * KTG : KTG, :],
                                    msb[:, lo - (NTx - QT) : hi - (NTx - QT), :],
                                )
                        for o in range(KTG):
                            kt = g * KTG + o
                            nc.tensor.matmul(
                                po[0:HD1, :],
                                lhsT=v[:, kt, h, :],
                                rhs=ex[:, o, :],
                                start=(g == 0 and o == 0),
                                stop=(g == NGx - 1 and o == KTG - 1),
                            )
                    dinv = dnp.tile([1, W], f32, tag="dinv")
                    nc.vector.reciprocal(dinv[0:1, :], po[HD:HD1, :])
                    dinvb = dnp.tile([HD, W], f32, tag="dinvb")
                    nc.gpsimd.partition_broadcast(
                        dinvb[0:HD, :], dinv[0:1, :], channels=HD
                    )
                    nc.vector.tensor_mul(
                        aoT[hh : hh + HD, h // HP, :],
                        po[0:HD, :],
                        dinvb[0:HD, :],
                    )

        # =========== layernorm (transposed layout, f32 internals) ===========
        def ln_t(pre, acc, sqa, out_bf, out_f, g_sb, b_sb, lpp, lp):
            """pre: [P, DT, W] f32 sbuf; acc/sqa already accumulated via
            ln_acc_step; out_bf bf16 (or None), out_f f32 (or None)."""
            sums = lpp.tile([1, W], f32, tag="lnsums")
            nc.tensor.matmul(sums[0:1, :], lhsT=ones_p1[:, :],
                             rhs=acc[:, :], start=True, stop=True)
            sqs = lpp.tile([1, W], f32, tag="lnsqs")
            nc.tensor.matmul(sqs[0:1, :], lhsT=ones_p1[:, :],
                             rhs=sqa[:, :], start=True, stop=True)
            mu = lp.tile([1, W], f32, tag="lnmu")
            nc.vector.tensor_scalar_mul(mu[0:1, :], sums[0:1, :], 1.0 / D)
            ex2 = lp.tile([1, W], f32, tag="lnex2")
            nc.vector.tensor_scalar_mul(ex2[0:1, :], sqs[0:1, :], 1.0 / D)
            mu2 = lp.tile([1, W], f32, tag="lnmu2")
            nc.scalar.square(mu2[0:1, :], mu[0:1, :])
            var = lp.tile([1, W], f32, tag="lnvar")
            nc.vector.tensor_sub(var[0:1, :], ex2[0:1, :], mu2[0:1, :])
            sd = lp.tile([1, W], f32, tag="lnsd")
            nc.scalar.activation(out=sd[0:1, :], in_=var[0:1, :], func=AF.Sqrt,
                                 bias=eps_t[0:1, :], scale=1.0)
            rstd = lp.tile([1, W], f32, tag="lnrstd")
            nc.vector.reciprocal(rstd[0:1, :], sd[0:1, :])
            mub = lpp.tile([P, W], f32, tag="lnmub")
            nc.tensor.matmul(mub[:, :], lhsT=ones_1p[0:1, :],
                             rhs=mu[0:1, :], start=True, stop=True)
            rstdb = lpp.tile([P, W], f32, tag="lnrstdb")
            nc.tensor.matmul(rstdb[:, :], lhsT=ones_1p[0:1, :],
                             rhs=rstd[0:1, :], start=True, stop=True)
            mubs = lp.tile([P, W], f32, tag="lnmubs")
            nc.vector.tensor_copy(mubs[:, :], mub[:, :])
            rstdbs = lp.tile([P, W], f32, tag="lnrstdbs")
            nc.vector.tensor_copy(rstdbs[:, :], rstdb[:, :])
            for d in range(DT):
                t1 = lp.tile([P, W], f32, tag="lnt1")
                nc.vector.tensor_sub(t1[:, :], pre[:, d, :], mubs[:, :])
                of = out_f[:, d, :] if out_f is not None else None
                if of is not None:
                    nc.vector.tensor_mul(of, t1[:, :], rstdbs[:, :])
                    if g_sb is not None:
                        nc.vector.tensor_scalar_mul(of, of, g_sb[:, d : d + 1])
                    if b_sb is not None:
                        nc.vector.tensor_scalar_add(of, of, b_sb[:, d : d + 1])
                    if out_bf is not None:
                        nc.vector.tensor_copy(out_bf[:, d, :], of)
                else:
                    ob = out_bf[:, d, :]
                    nc.vector.tensor_mul(ob, t1[:, :], rstdbs[:, :])
                    if g_sb is not None:
                        nc.vector.tensor_scalar_mul(ob, ob, g_sb[:, d : d + 1])
                    if b_sb is not None:
                        nc.vector.tensor_scalar_add(ob, ob, b_sb[:, d : d + 1])

        # ---- incremental LN stat accumulation (overlaps producer loops) ----
        def ln_acc_step(pre, d, acc, sqa, lp):
            if d == 0:
                nc.vector.tensor_copy(acc[:, :], pre[:, 0, :])
                nc.scalar.square(sqa[:, :], pre[:, 0, :])
            else:
                nc.vector.tensor_add(acc[:, :], acc[:, :], pre[:, d, :])
                sqt = lp.tile([P, W], f32, tag="lnsqt")
                nc.scalar.square(sqt[:, :], pre[:, d, :])
                nc.vector.tensor_add(sqa[:, :], sqa[:, :], sqt[:, :])

        # =========== out-projection + residual + LN ===========
        def proj_resid_ln(owT, obsb, residT, g_sb, b_sb, out_bf, out_f):
            with tc.tile_pool(name="pr_w", bufs=2) as wp, \
                 tc.tile_pool(name="pr_t", bufs=2) as lp, \
                 tc.tile_pool(name="pr_pre", bufs=1) as prep, \
                 tc.tile_pool(name="pr_ps", bufs=2, space="PSUM") as psp, \
                 tc.tile_pool(name="pr_lnps", bufs=1, space="PSUM") as lpp:
                pre = prep.tile([P, DT, W], f32, tag="pre")
                acc = lp.tile([P, W], f32, tag="lnacc")
                sqa = lp.tile([P, W], f32, tag="lnsqa")
                for dg in range(DT // G4):
                    wsl = wp.tile([P, DT, G4 * P], bf16, tag="prw")
                    nc.sync.dma_start(
                        out=wsl[:, :, :],
                        in_=owT[:, dg * G4 * P : (dg + 1) * G4 * P]
                        .rearrange("(t p) v -> p t v", p=P),
                    )
                    for j in range(G4):
                        d = dg * G4 + j
                        ps = psp.tile([P, W], f32, tag="prps")
                        for dt in range(DT):
                            nc.tensor.matmul(
                                ps[:, :], lhsT=wsl[:, dt, j * P : (j + 1) * P],
                                rhs=aoT[:, dt, :],
                                start=(dt == 0), stop=(dt == DT - 1),
                            )
                        if obsb is not None:
                            tmp = lp.tile([P, W], f32, tag="prtmp")
                            nc.scalar.activation(out=tmp[:, :], in_=ps[:, :],
                                                 func=AF.Identity,
                                                 bias=obsb[:, d : d + 1], scale=1.0)
                            nc.vector.tensor_add(pre[:, d, :], tmp[:, :],
                                                 residT[:, d, :])
                        else:
                            nc.vector.tensor_add(pre[:, d, :], ps[:, :],
                                                 residT[:, d, :])
                        ln_acc_step(pre, d, acc, sqa, lp)
                ln_t(pre, acc, sqa, out_bf, out_f, g_sb, b_sb, lpp, lp)

        # ================= block 1: self-attention =================
        with tc.tile_pool(name="kv1", bufs=1) as kvp1:
            kT1 = kvp1.tile([P, DT, S], bf16)
            v1 = kvp1.tile([P, NT, H, HD1], bf16)
            with tc.tile_pool(name="xs1", bufs=1) as xsp1:
                xs = xsp1.tile([P, DT, S], bf16)
                nc.sync.dma_start(out=xs[:, :, :],
                                  in_=xTr.rearrange("(t p) s -> p t s", p=P))
                # the host always rotates key order so this core's quarter
                # sits in the last QT slots (uniform across cores); any mask
                # data is supplied in rotated coordinates.
                xq_off = S - W
                xloc = xs[:, :, xq_off : xq_off + W]
                nc.vector.tensor_copy(xlocT[:, :, :], xloc)
                qkv_phase(xs, xloc, qkvwT1, kT1, v1, qkb1sb, vb1sb, zv1sb)
            attn_phase(kT1, v1, m1sb, m1fsb)
        proj_resid_ln(owT1, ob1sb, xlocT, lns["g1"], lns["b1"], x1T, xf)

        # ---- local K2/V2/Q2 from x1, then all-gather K2|V2 in group ----
        assert cfg.use_collective
        with tc.tile_pool(name="kvloc", bufs=1) as kvlp:
            klocT = kvlp.tile([P, DT, W], bf16)
            vloc = kvlp.tile([P, QT, H, HD1], bf16)
            qkv_phase(x1T, x1T, qkvwT2, klocT, vloc, qkb2sb, vb2sb, zv2sb)
            nc.sync.dma_start(
                out=kvs[0 : D * W].rearrange("(t p w) -> p t w", p=P, w=W),
                in_=klocT[:, :, :],
            )
            nc.sync.dma_start(
                out=kvs[D * W : LKV].rearrange("(q p h d) -> p q h d",
                                               p=P, h=H, d=HD1),
                in_=vloc[:, :, :, :],
            )
        if cfg.fake_gather:
            for g in range(4):
                nc.sync.dma_start(out=agkv[g * LKV : (g + 1) * LKV],
                                  in_=kvs[:])
        else:
            nc.gpsimd.collective_compute(
                "AllGather",
                bass.mybir.AluOpType.bypass,
                replica_groups=[[0, 1, 2, 3], [4, 5, 6, 7]],
                ins=[kvs[:]],
                outs=[agkv[:]],
            )

        # keep the PE's HAM clock warm across the ~130us collective stall:
        # slow fp32 matmuls into a scratch PSUM bank, no data deps, sized to
        # finish just before the gather lands so attention 2 starts at 2.4GHz
        # instead of re-ramping from 1.2GHz.
        WARMN = 120 if S >= 2048 else 8
        warm_src = const.tile([P, W], f32, tag="warmsrc")
        nc.vector.memset(warm_src[:, :], 0.0)
        with tc.tile_pool(name="warmps", bufs=1, space="PSUM") as wmp:
            wps = wmp.tile([1, W], f32, tag="warm")
            for _ in range(WARMN):
                nc.tensor.matmul(wps[0:1, :], lhsT=ones_p1[:, :],
                                 rhs=warm_src[:, :], start=True, stop=True)

        # ================= block 2: cross-attention =================
        with tc.tile_pool(name="kv2", bufs=1) as kvp2:
            kT2 = kvp2.tile([P, DT, S], bf16)
            v2 = kvp2.tile([P, NT, H, HD1], bf16)
            for g in range(4):
                nc.sync.dma_start(
                    out=kT2[:, :, g * W : (g + 1) * W],
                    in_=agkv[g * LKV : g * LKV + D * W]
                    .rearrange("(t p w) -> p t w", p=P, w=W),
                )
                nc.sync.dma_start(
                    out=v2[:, g * QT : (g + 1) * QT, :, :],
                    in_=agkv[g * LKV + D * W : (g + 1) * LKV]
                    .rearrange("(q p h d) -> p q h d", p=P, h=H, d=HD1),
                )
            attn_phase(kT2, v2, None, None)
        proj_resid_ln(owT2, ob2sb, xf, lns["g2"], lns["b2"], x2T, xf)

        # ================= FFN =================
        with tc.tile_pool(name="ffh", bufs=1) as fhp, \
             tc.tile_pool(name="ffw", bufs=2) as wp, \
             tc.tile_pool(name="fft", bufs=1) as lp, \
             tc.tile_pool(name="ffpre", bufs=1) as prep:
            hT = fhp.tile([P, FT, W], bf16)
            with tc.tile_pool(name="ffps1", bufs=2, space="PSUM") as psp:
                for fg in range(FT // G4):
                    wsl = wp.tile([P, DT, G4 * P], bf16, tag="f1w")
                    nc.sync.dma_start(
                        out=wsl[:, :, :],
                        in_=w1T[:, fg * G4 * P : (fg + 1) * G4 * P]
                        .rearrange("(t p) v -> p t v", p=P),
                    )
                    for j in range(G4):
                        f = fg * G4 + j
                        ps = psp.tile([P, W], f32, tag="f1ps")
                        for dt in range(DT):
                            nc.tensor.matmul(
                                ps[:, :], lhsT=wsl[:, dt, j * P : (j + 1) * P],
                                rhs=x2T[:, dt, :],
                                start=(dt == 0), stop=(dt == DT - 1),
                            )
                        if fb1sb is not None:
                            nc.scalar.activation(out=hT[:, f, :], in_=ps[:, :],
                                                 func=AF.Relu,
                                                 bias=fb1sb[:, f : f + 1], scale=1.0)
                        else:
                            nc.scalar.activation(out=hT[:, f, :], in_=ps[:, :],
                                                 func=AF.Relu)
            pre = prep.tile([P, DT, W], f32, tag="ffpre")
            acc = lp.tile([P, W], f32, tag="lnacc")
            sqa = lp.tile([P, W], f32, tag="lnsqa")
            FTC = min(8, FT)  # w2 staging chunk (ft tiles per DMA)
            with tc.tile_pool(name="ffps2", bufs=1, space="PSUM") as psq, \
                 tc.tile_pool(name="fflnps", bufs=1, space="PSUM") as lpp:
                for dg in range(DT // G4):
                    ps4 = []
                    for j in range(G4):
                        ps4j = psq.tile([P, W], f32, tag="f2ps%d" % j)
                        ps4.append(ps4j)
                    for fc in range(FT // FTC):
                        w2sl = wp.tile([P, FTC, G4 * P], bf16, tag="f2w")
                        nc.sync.dma_start(
                            out=w2sl[:, :, :],
                            in_=w2T[fc * FTC * P : (fc + 1) * FTC * P,
                                    dg * G4 * P : (dg + 1) * G4 * P]
                            .rearrange("(t p) v -> p t v", p=P),
                        )
                        for fo in range(FTC):
                            ft = fc * FTC + fo
                            for j in range(G4):
                                nc.tensor.matmul(
                                    ps4[j][:, :],
                                    lhsT=w2sl[:, fo, j * P : (j + 1) * P],
                                    rhs=hT[:, ft, :],
                                    start=(ft == 0), stop=(ft == FT - 1),
                                )
                    for j in range(G4):
                        d = dg * G4 + j
                        if fb2sb is not None:
                            tmp = lp.tile([P, W], f32, tag="f2tmp")
                            nc.scalar.activation(out=tmp[:, :], in_=ps4[j][:, :],
                                                 func=AF.Identity,
                                                 bias=fb2sb[:, d : d + 1], scale=1.0)
                            nc.vector.tensor_add(pre[:, d, :], tmp[:, :],
                                                 xf[:, d, :])
                        else:
                            nc.vector.tensor_add(pre[:, d, :], ps4[j][:, :],
                                                 xf[:, d, :])
                        ln_acc_step(pre, d, acc, sqa, lp)
                outp = prep.tile([P, DT, W], f32, tag="ffout")
                ln_t(pre, acc, sqa, None, outp, lns["g3"], lns["b3"], lpp, lp)
                nc.sync.dma_start(
                    out=out.rearrange("(t p) w -> p t w", p=P),
                    in_=outp[:, :, :])


def make_program(cfg, fl):
    from concourse import bacc
    import concourse.tile as tile

    nc = bacc.Bacc("TRN2", target_bir_lowering=False, debug=False,
                   num_devices=8)
    with tile.TileContext(nc) as tc:
        with nc.allow_low_precision(reason="bf16 kernel, rel-err gate 2e-2"):
            _build(nc, tc, cfg, fl)
    nc.compile()
    return nc


def prep_inputs(inputs, cfg):
    """Host-side data prep. Returns (in_maps, fl)."""
    import ml_dtypes

    bf = ml_dtypes.bfloat16
    B, S, D, H, DFF, W, NT, QT = (cfg.B, cfg.S, cfg.D, cfg.H, cfg.DFF,
                                  cfg.W, cfg.NT, cfg.QT)
    f = np.float32
    x = np.asarray(inputs["x"], f)
    enc = np.asarray(inputs["enc_out"])
    trg = np.asarray(inputs["trg_mask"])
    fl = Flags()
    fl.qkb1 = bool(np.any(inputs["qkv_b1"]))
    fl.qkb2 = bool(np.any(inputs["qkv_b2"]))
    fl.vb1 = bool(np.any(np.asarray(inputs["qkv_b1"])[2 * D :]))
    fl.vb2 = bool(np.any(np.asarray(inputs["qkv_b2"])[2 * D :]))
    fl.ob1 = bool(np.any(inputs["out_b1"]))
    fl.ob2 = bool(np.any(inputs["out_b2"]))
    fl.fb1 = bool(np.any(inputs["ff_b1"]))
    fl.fb2 = bool(np.any(inputs["ff_b2"]))
    fl.g1 = not bool(np.all(np.asarray(inputs["ln1_g"]) == 1))
    fl.b1 = bool(np.any(inputs["ln1_b"]))
    fl.g2 = not bool(np.all(np.asarray(inputs["ln2_g"]) == 1))
    fl.b2 = bool(np.any(inputs["ln2_b"]))
    fl.g3 = not bool(np.all(np.asarray(inputs["ln3_g"]) == 1))
    fl.b3 = bool(np.any(inputs["ln3_b"]))
    fl.m1 = not bool(np.all(trg != 0))
    tril = np.tril(np.ones((S, S), np.int32))
    is_tril = (trg.shape[0] == 1 and
               bool(np.array_equal((trg[0, 0] != 0).astype(np.int32), tril)))
    fl.m1full = fl.m1 and not is_tril
    fl.zv2 = bool(np.any(enc == 0))

    def bcast(a):
        return np.ascontiguousarray(np.asarray(a, f).T.astype(bf))

    shared = {
        "qkvwT1": bcast(inputs["qkv_w1"]),
        "qkvwT2": bcast(inputs["qkv_w2"]),
        "owT1": bcast(inputs["out_w1"]),
        "owT2": bcast(inputs["out_w2"]),
        "w1T": bcast(inputs["ff_w1"]),
        "w2T": bcast(inputs["ff_w2"]),
    }
    if fl.qkb1:
        shared["qkvb1"] = np.asarray(inputs["qkv_b1"], f)
    if fl.qkb2:
        shared["qkvb2"] = np.asarray(inputs["qkv_b2"], f)
    if fl.vb1:
        shared["vb1"] = np.broadcast_to(
            np.asarray(inputs["qkv_b1"], f)[2 * D :], (P, D)).astype(bf)
    if fl.vb2:
        shared["vb2"] = np.broadcast_to(
            np.asarray(inputs["qkv_b2"], f)[2 * D :], (P, D)).astype(bf)
    if fl.ob1:
        shared["ob1"] = np.asarray(inputs["out_b1"], f)
    if fl.ob2:
        shared["ob2"] = np.asarray(inputs["out_b2"], f)
    if fl.fb1:
        shared["fb1"] = np.asarray(inputs["ff_b1"], f)
    if fl.fb2:
        shared["fb2"] = np.asarray(inputs["ff_b2"], f)
    for nm, key, use in [("g1", "ln1_g", fl.g1), ("b1", "ln1_b", fl.b1),
                         ("g2", "ln2_g", fl.g2), ("b2", "ln2_b", fl.b2),
                         ("g3", "ln3_g", fl.g3), ("b3", "ln3_b", fl.b3)]:
        if use:
            shared[nm] = np.asarray(inputs[key], f)
    if fl.m1 and not fl.m1full:
        # constant triangular mask for the last QT slots, same on all cores:
        # m1c[k', s, q'] = 1 if s*P + k' <= q'
        kk = np.arange(P)[:, None, None]
        ss = np.arange(QT)[None, :, None]
        qq = np.arange(W)[None, None, :]
        shared["m1c"] = ((ss * P + kk) <= qq).astype(bf)

    xTb = [np.ascontiguousarray(x[b].T).astype(bf) for b in range(B)]
    in_maps = []
    for c in range(8):
        b, r = c // 4, c % 4
        m = dict(shared)
        # rotate key tiles: slot t holds physical tile p(t) = (t+(r+1)*QT)%NT
        perm = [(t + (r + 1) * QT) % NT for t in range(NT)]
        xt = xTb[b].reshape(D, NT, P)
        m["xTr"] = np.ascontiguousarray(
            xt[:, perm, :].reshape(D, S))
        if fl.m1 and not fl.m1full:
            zv = np.zeros((P, NT), f)
            for t in range(NT):
                if perm[t] < (r + 1) * QT:
                    zv[:, t] = 1
            m["zv1"] = zv
        if fl.m1full:
            # generic multiplicative mask in rotated key coordinates
            tb = trg[b] if trg.shape[0] == B else trg[0]
            blk = (tb[0, r * W : (r + 1) * W, :] != 0).astype(f)  # [W, S](q,k)
            mk = blk.T.reshape(NT, P, W)  # [kt, k', q]
            m["m1f"] = np.ascontiguousarray(mk[perm]).astype(bf)
        if fl.zv2:
            # this core's own quarter only (keys it contributes to the
            # gathered V2) -- applied before the gather, so consumers see
            # already-zeroed rows
            eb = (np.asarray(enc)[b, 0, 0, r * W : (r + 1) * W] != 0).astype(f)
            m["zv2"] = np.ascontiguousarray(
                eb.reshape(W // P, P).T).astype(f)
        in_maps.append(m)
    return in_maps, fl


def kernel_with_results(**inputs):
    from concourse.bass_utils import run_bass_kernel_spmd

    cfg = Cfg()
    x = np.asarray(inputs["x"])
    assert x.shape == (cfg.B, cfg.S, cfg.D), x.shape
    in_maps, fl = prep_inputs(inputs, cfg)
    nc = make_program(cfg, fl)
    res = run_bass_kernel_spmd(nc, in_maps, list(range(8)))
    y = np.empty((cfg.B, cfg.S, cfg.D), np.float32)
    for c in range(8):
        b, r = c // 4, c % 4
        y[b, r * cfg.W : (r + 1) * cfg.W, :] = res.results[c]["out"].T
    return y, res


def kernel(**inputs):
    return kernel_with_results(**inputs)[0]


# revision 34
# speedup vs baseline: 1.3531x; 1.0288x over previous
"""Trainium2 Bass kernel for nn_DecoderLayer (dense transformer decoder layer).

Strategy (8 NeuronCores, full inputs in / full output out):
  - core c handles batch b = c//4 and query-quarter r = c%4 (rows [r*S/4, (r+1)*S/4)).
  - All matmul operands are bf16 (PSUM accumulation fp32); activations are
    kept TRANSPOSED on-chip (x^T [D, n]) so projections run with the
    contraction dim on partitions.
  - K^T / V / Q^T live entirely in SBUF (no HBM round-trip).
  - Self-attention causal masking: the host rotates each core's key order so
    the 4 "diagonal" key tiles sit at fixed slots (last group); keys fully
    below the diagonal get their V rows zeroed at projection time (per-key
    0/1 scale fused into the PSUM evacuation), so only the last exp group
    needs a (core-independent, constant) triangular multiplicative mask.
  - Cross-attention key masking (enc mask) uses the same V-row zeroing.
  - Softmax denominators come free from a ones column appended to V.
  - The single collective: AllGather of x1 (post-LN1) within each 4-core
    batch group, needed because cross-attention K2/V2 are projections of the
    full x1.
  - LayerNorm runs in transposed layout: cross-partition sums via ones-matmul
    on the PE, stats broadcast back to [128, W] via ones-matmul.
"""

import sys

if "/opt/trn_rl_repo" not in sys.path:
    sys.path.insert(0, "/opt/trn_rl_repo")

import numpy as np

P = 128
HD = 64
HD1 = HD + 1
EPS = 1e-5


class Cfg:
    def __init__(self, B=2, S=2048, D=1024, H=16, DFF=4096, use_collective=True,
                 fake_gather=False):
        self.B, self.S, self.D, self.H, self.DFF = B, S, D, H, DFF
        self.fake_gather = fake_gather
        self.W = S // 4            # local query rows per core
        self.DT = D // P           # feature-dim tiles
        self.NT = S // P           # sequence tiles (keys)
        self.FT = DFF // P         # ffn hidden tiles
        self.HP = P // HD          # heads per partition-tile (2)
        self.QT = self.W // P      # key tiles per query quarter (diag tiles)
        self.KTG = min(2, self.NT)    # k-tiles per exp group
        self.NG = self.NT // self.KTG
        self.VCW = min(512, D)        # v-dout chunk width
        self.VCN = D // self.VCW
        self.HPC = self.VCW // HD     # heads per v-chunk
        self.KCW = min(512, self.S)   # k-proj token chunk width
        self.KCN = self.S // self.KCW
        self.G4 = min(4, self.DT)
        self.use_collective = use_collective
        assert D == H * HD
        assert self.W % P == 0 and D % P == 0 and DFF % P == 0 and S % P == 0


class Flags:
    def __init__(self):
        self.qkb1 = self.vb1 = self.ob1 = False
        self.qkb2 = self.vb2 = self.ob2 = False
        self.fb1 = self.fb2 = False
        self.g1 = self.b1 = self.g2 = self.b2 = self.g3 = self.b3 = False
        self.m1 = True      # trg mask active (tril fast path)
        self.m1full = False  # generic (non-tril) trg mask: full mult tiles
        self.zv2 = False    # enc mask active -> zero V2 rows


def _build(nc, tc, cfg, fl):
    import concourse.bass as bass
    import concourse.mybir as mybir
    import concourse.tile as tile  # noqa: F401
    from contextlib import ExitStack

    AF = mybir.ActivationFunctionType
    f32 = mybir.dt.float32
    bf16 = mybir.dt.bfloat16

    B, S, D, H, DFF = cfg.B, cfg.S, cfg.D, cfg.H, cfg.DFF
    W, DT, NT, FT, HP = cfg.W, cfg.DT, cfg.NT, cfg.FT, cfg.HP
    QT, KTG, NG = cfg.QT, cfg.KTG, cfg.NG
    VCW, VCN, HPC = cfg.VCW, cfg.VCN, cfg.HPC
    KCW, KCN, G4 = cfg.KCW, cfg.KCN, cfg.G4

    # ---------------- DRAM parameters ----------------
    def din(name, shape, dt=bf16):
        return nc.dram_tensor(name, shape, dt, kind="ExternalInput").ap()

    xTr = din("xTr", [D, S])          # rotated x^T for this core
    qkvwT1 = din("qkvwT1", [D, 3 * D])
    qkvwT2 = din("qkvwT2", [D, 3 * D])
    owT1 = din("owT1", [D, D])
    owT2 = din("owT2", [D, D])
    w1T = din("w1T", [D, DFF])
    w2T = din("w2T", [DFF, D])
    m1c = din("m1c", [P, QT, W]) if (fl.m1 and not fl.m1full) else None
    m1f = din("m1f", [NT, P, W]) if fl.m1full else None
    zv1 = din("zv1", [P, NT], f32) if fl.m1 and not fl.m1full else None
    zv2d = din("zv2", [P, QT], f32) if fl.zv2 else None
    qkvb1 = din("qkvb1", [3 * D], f32) if fl.qkb1 else None
    qkvb2 = din("qkvb2", [3 * D], f32) if fl.qkb2 else None
    vb1 = din("vb1", [P, D]) if fl.vb1 else None
    vb2 = din("vb2", [P, D]) if fl.vb2 else None
    ob1 = din("ob1", [D], f32) if fl.ob1 else None
    ob2 = din("ob2", [D], f32) if fl.ob2 else None
    fb1d = din("fb1", [DFF], f32) if fl.fb1 else None
    fb2d = din("fb2", [D], f32) if fl.fb2 else None
    lnp = {}
    for nm, use in [("g1", fl.g1), ("b1", fl.b1), ("g2", fl.g2),
                    ("b2", fl.b2), ("g3", fl.g3), ("b3", fl.b3)]:
        lnp[nm] = din(nm, [D], f32) if use else None
    out = nc.dram_tensor("out", [D, W], f32, kind="ExternalOutput").ap()

    es = ExitStack()
    with es:
        dramp = es.enter_context(tc.tile_pool(name="dram", bufs=1, space="DRAM"))
        LKV = D * W + W * H * HD1  # flat K2loc + V2loc staging elements
        if cfg.use_collective:
            kvs = dramp.tile([LKV], bf16)
            agkv = dramp.tile([4 * LKV], bf16)

        const = es.enter_context(tc.tile_pool(name="const", bufs=1))
        ones_p1 = const.tile([P, 1], f32)
        nc.vector.memset(ones_p1[:, :], 1.0)
        ones_1p = const.tile([1, P], f32)
        nc.vector.memset(ones_1p[0:1, :], 1.0)
        ones_col = const.tile([P, HPC, 1], bf16)
        nc.vector.memset(ones_col[:, :, :], 1.0)
        eps_t = const.tile([1, 1], f32)
        nc.vector.memset(eps_t[0:1, :], EPS)

        def ldvec(dram_vec, n_tiles, name):
            """[D]-style f32 vector -> [P, n_tiles] sbuf tile."""
            t = const.tile([P, n_tiles], f32, tag=name)
            nc.sync.dma_start(
                out=t[:, :],
                in_=dram_vec.rearrange("(t p) -> p t", p=P),
            )
            return t

        qkb1sb = ldvec(qkvb1[0 : 2 * D], 2 * DT, "qkb1") if fl.qkb1 else None
        qkb2sb = ldvec(qkvb2[0 : 2 * D], 2 * DT, "qkb2") if fl.qkb2 else None
        ob1sb = ldvec(ob1, DT, "ob1") if fl.ob1 else None
        ob2sb = ldvec(ob2, DT, "ob2") if fl.ob2 else None
        fb1sb = ldvec(fb1d, FT, "fb1") if fl.fb1 else None
        fb2sb = ldvec(fb2d, DT, "fb2") if fl.fb2 else None
        lns = {k: (ldvec(v, DT, "ln" + k) if v is not None else None)
               for k, v in lnp.items()}
        vb1sb = None
        if fl.vb1:
            vb1sb = const.tile([P, D], bf16, tag="vb1")
            nc.sync.dma_start(out=vb1sb[:, :], in_=vb1[:, :])
        vb2sb = None
        if fl.vb2:
            vb2sb = const.tile([P, D], bf16, tag="vb2")
            nc.sync.dma_start(out=vb2sb[:, :], in_=vb2[:, :])
        zv1sb = None
        if zv1 is not None:
            zv1sb = const.tile([P, NT], f32, tag="zv1")
            nc.sync.dma_start(out=zv1sb[:, :], in_=zv1[:, :])
        zv2sb = None
        if zv2d is not None:
            zv2sb = const.tile([P, QT], f32, tag="zv2")
            nc.sync.dma_start(out=zv2sb[:, :], in_=zv2d[:, :])
        m1sb = None
        if m1c is not None:
            m1sb = const.tile([P, QT, W], bf16, tag="m1c")
            nc.sync.dma_start(out=m1sb[:, :, :], in_=m1c[:, :, :])
        m1fsb = None
        if m1f is not None:
            m1fsb = const.tile([P, NT, W], bf16, tag="m1f")
            nc.sync.dma_start(out=m1fsb[:, :, :], in_=m1f.rearrange("n p w -> p n w"))

        # persistent mid tensors
        midp = es.enter_context(tc.tile_pool(name="mid", bufs=1))
        qT = midp.tile([P, DT, W], bf16)      # Q^T local (reused block2)
        aoT = midp.tile([P, DT, W], bf16)     # attention out^T (reused)
        x1T = midp.tile([P, DT, W], bf16)     # x1 local
        x2T = midp.tile([P, DT, W], bf16)     # x2 local
        xlocT = midp.tile([P, DT, W], bf16)   # this core's x quarter (resid 1)
        xf = midp.tile([P, DT, W], f32)       # f32 residual backbone (x1, x2)

        # =========== QKV projection (into SBUF K/V/Q) ===========
        def qkv_phase(xsb, xq, wT, kT, v, qkb, vbsb, zvsb):
            """xsb: [P, DT, Sx] bf16 x^T source (tokens = key order);
            xq: [P, DT, W] bf16 x^T source for this core's queries;
            writes kT [P, DT, Sx] sbuf, v [P, NT_x, H, HD1] sbuf, qT."""
            Sx = xsb.shape[2]
            NTx = Sx // P
            KCWx = min(KCW, Sx)
            KCNx = Sx // KCWx
            with tc.tile_pool(name="qkv_w", bufs=2) as wp, \
                 tc.tile_pool(name="qkv_ps", bufs=2, space="PSUM") as psp:
                # ---- K^T ----
                wall = wp.tile([P, DT, D], bf16, tag="wall")
                nc.sync.dma_start(
                    out=wall[:, :, :],
                    in_=wT[:, D : 2 * D].rearrange("(t p) v -> p t v", p=P),
                )
                for nch in range(KCNx):
                    for dk in range(DT):
                        ps = psp.tile([P, KCWx], f32, tag="kps")
                        for dt in range(DT):
                            nc.tensor.matmul(
                                ps[:, :],
                                lhsT=wall[:, dt, dk * P : (dk + 1) * P],
                                rhs=xsb[:, dt, nch * KCWx : (nch + 1) * KCWx],
                                start=(dt == 0),
                                stop=(dt == DT - 1),
                            )
                        if qkb is not None:
                            nc.scalar.activation(
                                out=kT[:, dk, nch * KCWx : (nch + 1) * KCWx],
                                in_=ps[:, :], func=AF.Identity,
                                bias=qkb[:, DT + dk : DT + dk + 1], scale=1.0,
                            )
                        else:
                            nc.scalar.activation(
                                out=kT[:, dk, nch * KCWx : (nch + 1) * KCWx],
                                in_=ps[:, :], func=AF.Copy,
                            )
                # ---- Q^T local [D, W] ----
                wall = wp.tile([P, DT, D], bf16, tag="wall")
                nc.sync.dma_start(
                    out=wall[:, :, :],
                    in_=wT[:, 0:D].rearrange("(t p) v -> p t v", p=P),
                )
                for dq in range(DT):
                    ps = psp.tile([P, W], f32, tag="qps")
                    for dt in range(DT):
                        nc.tensor.matmul(
                            ps[:, :],
                            lhsT=wall[:, dt, dq * P : (dq + 1) * P],
                            rhs=xq[:, dt, :],
                            start=(dt == 0),
                            stop=(dt == DT - 1),
                        )
                    if qkb is not None:
                        nc.scalar.activation(
                            out=qT[:, dq, :], in_=ps[:, :], func=AF.Identity,
                            bias=qkb[:, dq : dq + 1], scale=1.0,
                        )
                    else:
                        nc.scalar.activation(
                            out=qT[:, dq, :], in_=ps[:, :], func=AF.Copy,
                        )
                # ---- V natural [n, dout] + ones column ----
                wall = wp.tile([P, DT, D], bf16, tag="wall")
                nc.sync.dma_start(
                    out=wall[:, :, :],
                    in_=wT[:, 2 * D : 3 * D].rearrange("(t p) v -> p t v", p=P),
                )
                for nt in range(NTx):
                    for vc in range(VCN):
                        ps = psp.tile([P, VCW], f32, tag="vps")
                        for dt in range(DT):
                            nc.tensor.matmul(
                                ps[:, :],
                                lhsT=xsb[:, dt, nt * P : (nt + 1) * P],
                                rhs=wall[:, dt, vc * VCW : (vc + 1) * VCW],
                                start=(dt == 0),
                                stop=(dt == DT - 1),
                            )
                        dst = v[:, nt, vc * HPC : (vc + 1) * HPC, 0:HD]
                        psv = ps.rearrange("p (h d) -> p h d", d=HD)
                        if vbsb is not None:
                            # bias first, then per-key zeroing (mask applies
                            # to the biased value)
                            nc.scalar.activation(out=dst, in_=psv, func=AF.Copy)
                            nc.vector.tensor_add(
                                dst, dst,
                                vbsb[:, vc * VCW : (vc + 1) * VCW].rearrange(
                                    "p (h d) -> p h d", d=HD),
                            )
                            if zvsb is not None:
                                nc.vector.tensor_scalar_mul(
                                    dst, dst, zvsb[:, nt : nt + 1])
                        elif zvsb is not None:
                            nc.scalar.activation(
                                out=dst, in_=psv, func=AF.Copy,
                                scale=zvsb[:, nt : nt + 1],
                            )
                        else:
                            nc.scalar.activation(out=dst, in_=psv, func=AF.Copy)
                        oc = v[:, nt, vc * HPC : (vc + 1) * HPC, HD:HD1]
                        if zvsb is not None:
                            nc.scalar.activation(
                                out=oc, in_=ones_col[:, :, :], func=AF.Copy,
                                scale=zvsb[:, nt : nt + 1],
                            )
                        else:
                            nc.vector.memset(oc, 1.0)

        # =========== attention phase ===========
        def attn_phase(kT, v, msb, mfull):
            """msb: [P, QT, W] triangular mask on the LAST QT slots (or None);
            mfull: [P, NT, W] generic multiplicative mask (or None).
            Score PSUM is manually double-buffered (two KTG-slot halves of one
            [P, 2*KTG, W] tile) so the next group's matmuls never wait on the
            previous group's exp."""
            NTx = v.shape[1]
            NGx = NTx // KTG
            with tc.tile_pool(name="at_ex", bufs=3) as exp_, \
                 tc.tile_pool(name="at_dn", bufs=2) as dnp, \
                 tc.tile_pool(name="at_ps", bufs=2, space="PSUM") as psp, \
                 tc.tile_pool(name="at_po", bufs=2, space="PSUM") as pop, \
                 tc.tile_pool(name="at_wm", bufs=1, space="PSUM") as wmp:
                # attention matmuls use at most half the PE array (64-row
                # contraction for scores, 65-col output for AV), which the
                # HAM activity monitor does not count as busy -- so the PE
                # clock stays at whatever state attention entered with. A
                # full-array 128x128 matmul interleaved every other group
                # keeps tripping the monitor so the array runs at 2.4GHz.
                wps = wmp.tile([P, W], f32, tag="warm")
                for h in range(H):
                    hh = (h % HP) * HD
                    q_h = qT[hh : hh + HD, h // HP, :]
                    po = pop.tile([P, W], f32, tag="po")
                    pstiles = {}

                    def scores(g):
                        ps = psp.tile([P, KTG, W], f32, tag="sc")
                        pstiles[g] = ps
                        for o in range(KTG):
                            kt = g * KTG + o
                            nc.tensor.matmul(
                                ps[:, o, :],
                                lhsT=kT[hh : hh + HD, h // HP,
                                        kt * P : (kt + 1) * P],
                                rhs=q_h,
                                start=True,
                                stop=True,
                            )

                    # software-pipelined: scores(g+1) is issued to the PE
                    # ahead of AV(g), so the PE never sits behind an AV that
                    # is itself waiting on exp(g).
                    scores(0)
                    for g in range(NGx):
                        if g + 1 < NGx:
                            scores(g + 1)
                        if g % 2 == 0:
                            nc.tensor.matmul(
                                wps[:, :], lhsT=kT[:, 0, 0:P],
                                rhs=kT[:, 0, 0:W], start=True, stop=True,
                            )
                        ps = pstiles.pop(g)
                        ex = exp_.tile([P, KTG, W], bf16, tag="ex")
                        nc.scalar.activation(
                            out=ex[:, :, :], in_=ps[:, :, :], func=AF.Exp,
                            scale=1.0 / float(np.sqrt(HD)),
                        )
                        if mfull is not None:
                            nc.vector.tensor_mul(
                                ex[:, :, :], ex[:, :, :],
                                mfull[:, g * KTG : (g + 1) * KTG, :],
                            )
                        elif msb is not None:
                            # overlap of this group's slots with the diagonal
                            # region [NTx-QT, NTx)
                            lo = max(g * KTG, NTx - QT)
                            hi = (g + 1) * KTG
                            if lo < hi:
                                nc.vector.tensor_mul(
                                    ex[:, lo - g * KTG : KTG, :],
                                    ex[:, lo - g * KTG : KTG, :],
                                    msb[:, lo - (NTx - QT) : hi - (NTx - QT), :],
                                )
                        for o in range(KTG):
                            kt = g * KTG + o
                            nc.tensor.matmul(
                                po[0:HD1, :],
                                lhsT=v[:, kt, h, :],
                                rhs=ex[:, o, :],
                                start=(g == 0 and o == 0),
                                stop=(g == NGx - 1 and o == KTG - 1),
                            )
                    dinv = dnp.tile([1, W], f32, tag="dinv")
                    nc.vector.reciprocal(dinv[0:1, :], po[HD:HD1, :])
                    dinvb = dnp.tile([HD, W], f32, tag="dinvb")
                    nc.gpsimd.partition_broadcast(
                        dinvb[0:HD, :], dinv[0:1, :], channels=HD
                    )
                    nc.vector.tensor_mul(
                        aoT[hh : hh + HD, h // HP, :],
                        po[0:HD, :],
                        dinvb[0:HD, :],
                    )

        # =========== layernorm (transposed layout, f32 internals) ===========
        def ln_t(pre, acc, sqa, out_bf, out_f, g_sb, b_sb, lpp, lp):
            """pre: [P, DT, W] f32 sbuf; acc/sqa already accumulated via
            ln_acc_step; out_bf bf16 (or None), out_f f32 (or None)."""
            sums = lpp.tile([1, W], f32, tag="lnsums")
            nc.tensor.matmul(sums[0:1, :], lhsT=ones_p1[:, :],
                             rhs=acc[:, :], start=True, stop=True)
            sqs = lpp.tile([1, W], f32, tag="lnsqs")
            nc.tensor.matmul(sqs[0:1, :], lhsT=ones_p1[:, :],
                             rhs=sqa[:, :], start=True, stop=True)
            mu = lp.tile([1, W], f32, tag="lnmu")
            nc.vector.tensor_scalar_mul(mu[0:1, :], sums[0:1, :], 1.0 / D)
            ex2 = lp.tile([1, W], f32, tag="lnex2")
            nc.vector.tensor_scalar_mul(ex2[0:1, :], sqs[0:1, :], 1.0 / D)
            mu2 = lp.tile([1, W], f32, tag="lnmu2")
            nc.scalar.square(mu2[0:1, :], mu[0:1, :])
            var = lp.tile([1, W], f32, tag="lnvar")
            nc.vector.tensor_sub(var[0:1, :], ex2[0:1, :], mu2[0:1, :])
            sd = lp.tile([1, W], f32, tag="lnsd")
            nc.scalar.activation(out=sd[0:1, :], in_=var[0:1, :], func=AF.Sqrt,
                                 bias=eps_t[0:1, :], scale=1.0)
            rstd = lp.tile([1, W], f32, tag="lnrstd")
            nc.vector.reciprocal(rstd[0:1, :], sd[0:1, :])
            mub = lpp.tile([P, W], f32, tag="lnmub")
            nc.tensor.matmul(mub[:, :], lhsT=ones_1p[0:1, :],
                             rhs=mu[0:1, :], start=True, stop=True)
            rstdb = lpp.tile([P, W], f32, tag="lnrstdb")
            nc.tensor.matmul(rstdb[:, :], lhsT=ones_1p[0:1, :],
                             rhs=rstd[0:1, :], start=True, stop=True)
            mubs = lp.tile([P, W], f32, tag="lnmubs")
            nc.vector.tensor_copy(mubs[:, :], mub[:, :])
            rstdbs = lp.tile([P, W], f32, tag="lnrstdbs")
            nc.vector.tensor_copy(rstdbs[:, :], rstdb[:, :])
            for d in range(DT):
                t1 = lp.tile([P, W], f32, tag="lnt1")
                nc.vector.tensor_sub(t1[:, :], pre[:, d, :], mubs[:, :])
                of = out_f[:, d, :] if out_f is not None else None
                if of is not None:
                    nc.vector.tensor_mul(of, t1[:, :], rstdbs[:, :])
                    if g_sb is not None:
                        nc.vector.tensor_scalar_mul(of, of, g_sb[:, d : d + 1])
                    if b_sb is not None:
                        nc.vector.tensor_scalar_add(of, of, b_sb[:, d : d + 1])
                    if out_bf is not None:
                        nc.vector.tensor_copy(out_bf[:, d, :], of)
                else:
                    ob = out_bf[:, d, :]
                    nc.vector.tensor_mul(ob, t1[:, :], rstdbs[:, :])
                    if g_sb is not None:
                        nc.vector.tensor_scalar_mul(ob, ob, g_sb[:, d : d + 1])
                    if b_sb is not None:
                        nc.vector.tensor_scalar_add(ob, ob, b_sb[:, d : d + 1])

        # ---- incremental LN stat accumulation (overlaps producer loops) ----
        def ln_acc_step(pre, d, acc, sqa, lp):
            if d == 0:
                nc.vector.tensor_copy(acc[:, :], pre[:, 0, :])
                nc.scalar.square(sqa[:, :], pre[:, 0, :])
            else:
                nc.vector.tensor_add(acc[:, :], acc[:, :], pre[:, d, :])
                sqt = lp.tile([P, W], f32, tag="lnsqt")
                nc.scalar.square(sqt[:, :], pre[:, d, :])
                nc.vector.tensor_add(sqa[:, :], sqa[:, :], sqt[:, :])

        # =========== out-projection + residual + LN ===========
        def proj_resid_ln(owT, obsb, residT, g_sb, b_sb, out_bf, out_f):
            with tc.tile_pool(name="pr_w", bufs=2) as wp, \
                 tc.tile_pool(name="pr_t", bufs=2) as lp, \
                 tc.tile_pool(name="pr_pre", bufs=1) as prep, \
                 tc.tile_pool(name="pr_ps", bufs=2, space="PSUM") as psp, \
                 tc.tile_pool(name="pr_lnps", bufs=1, space="PSUM") as lpp:
                pre = prep.tile([P, DT, W], f32, tag="pre")
                acc = lp.tile([P, W], f32, tag="lnacc")
                sqa = lp.tile([P, W], f32, tag="lnsqa")
                for dg in range(DT // G4):
                    wsl = wp.tile([P, DT, G4 * P], bf16, tag="prw")
                    nc.sync.dma_start(
                        out=wsl[:, :, :],
                        in_=owT[:, dg * G4 * P : (dg + 1) * G4 * P]
                        .rearrange("(t p) v -> p t v", p=P),
                    )
                    for j in range(G4):
                        d = dg * G4 + j
                        ps = psp.tile([P, W], f32, tag="prps")
                        for dt in range(DT):
                            nc.tensor.matmul(
                                ps[:, :], lhsT=wsl[:, dt, j * P : (j + 1) * P],
                                rhs=aoT[:, dt, :],
                                start=(dt == 0), stop=(dt == DT - 1),
                            )
                        if obsb is not None:
                            tmp = lp.tile([P, W], f32, tag="prtmp")
                            nc.scalar.activation(out=tmp[:, :], in_=ps[:, :],
                                                 func=AF.Identity,
                                                 bias=obsb[:, d : d + 1], scale=1.0)
                            nc.vector.tensor_add(pre[:, d, :], tmp[:, :],
                                                 residT[:, d, :])
                        else:
                            nc.vector.tensor_add(pre[:, d, :], ps[:, :],
                                                 residT[:, d, :])
                        ln_acc_step(pre, d, acc, sqa, lp)
                ln_t(pre, acc, sqa, out_bf, out_f, g_sb, b_sb, lpp, lp)

        # ================= block 1: self-attention =================
        with tc.tile_pool(name="kv1", bufs=1) as kvp1:
            kT1 = kvp1.tile([P, DT, S], bf16)
            v1 = kvp1.tile([P, NT, H, HD1], bf16)
            with tc.tile_pool(name="xs1", bufs=1) as xsp1:
                xs = xsp1.tile([P, DT, S], bf16)
                nc.sync.dma_start(out=xs[:, :, :],
                                  in_=xTr.rearrange("(t p) s -> p t s", p=P))
                # the host always rotates key order so this core's quarter
                # sits in the last QT slots (uniform across cores); any mask
                # data is supplied in rotated coordinates.
                xq_off = S - W
                xloc = xs[:, :, xq_off : xq_off + W]
                nc.vector.tensor_copy(xlocT[:, :, :], xloc)
                qkv_phase(xs, xloc, qkvwT1, kT1, v1, qkb1sb, vb1sb, zv1sb)
            attn_phase(kT1, v1, m1sb, m1fsb)
        proj_resid_ln(owT1, ob1sb, xlocT, lns["g1"], lns["b1"], x1T, xf)

        # ---- local K2/V2/Q2 from x1, then all-gather K2|V2 in group ----
        assert cfg.use_collective
        with tc.tile_pool(name="kvloc", bufs=1) as kvlp:
            klocT = kvlp.tile([P, DT, W], bf16)
            vloc = kvlp.tile([P, QT, H, HD1], bf16)
            qkv_phase(x1T, x1T, qkvwT2, klocT, vloc, qkb2sb, vb2sb, zv2sb)
            nc.sync.dma_start(
                out=kvs[0 : D * W].rearrange("(t p w) -> p t w", p=P, w=W),
                in_=klocT[:, :, :],
            )
            nc.sync.dma_start(
                out=kvs[D * W : LKV].rearrange("(q p h d) -> p q h d",
                                               p=P, h=H, d=HD1),
                in_=vloc[:, :, :, :],
            )
        if cfg.fake_gather:
            for g in range(4):
                nc.sync.dma_start(out=agkv[g * LKV : (g + 1) * LKV],
                                  in_=kvs[:])
        else:
            nc.gpsimd.collective_compute(
                "AllGather",
                bass.mybir.AluOpType.bypass,
                replica_groups=[[0, 1, 2, 3], [4, 5, 6, 7]],
                ins=[kvs[:]],
                outs=[agkv[:]],
            )



        # ================= block 2: cross-attention =================
        with tc.tile_pool(name="kv2", bufs=1) as kvp2:
            kT2 = kvp2.tile([P, DT, S], bf16)
            v2 = kvp2.tile([P, NT, H, HD1], bf16)
            for g in range(4):
                nc.sync.dma_start(
                    out=kT2[:, :, g * W : (g + 1) * W],
                    in_=agkv[g * LKV : g * LKV + D * W]
                    .rearrange("(t p w) -> p t w", p=P, w=W),
                )
                nc.sync.dma_start(
                    out=v2[:, g * QT : (g + 1) * QT, :, :],
                    in_=agkv[g * LKV + D * W : (g + 1) * LKV]
                    .rearrange("(q p h d) -> p q h d", p=P, h=H, d=HD1),
                )
            attn_phase(kT2, v2, None, None)
        proj_resid_ln(owT2, ob2sb, xf, lns["g2"], lns["b2"], x2T, xf)

        # ================= FFN =================
        with tc.tile_pool(name="ffh", bufs=1) as fhp, \
             tc.tile_pool(name="ffw", bufs=2) as wp, \
             tc.tile_pool(name="fft", bufs=1) as lp, \
             tc.tile_pool(name="ffpre", bufs=1) as prep:
            hT = fhp.tile([P, FT, W], bf16)
            with tc.tile_pool(name="ffps1", bufs=2, space="PSUM") as psp:
                for fg in range(FT // G4):
                    wsl = wp.tile([P, DT, G4 * P], bf16, tag="f1w")
                    nc.sync.dma_start(
                        out=wsl[:, :, :],
                        in_=w1T[:, fg * G4 * P : (fg + 1) * G4 * P]
                        .rearrange("(t p) v -> p t v", p=P),
                    )
                    for j in range(G4):
                        f = fg * G4 + j
                        ps = psp.tile([P, W], f32, tag="f1ps")
                        for dt in range(DT):
                            nc.tensor.matmul(
                                ps[:, :], lhsT=wsl[:, dt, j * P : (j + 1) * P],
                                rhs=x2T[:, dt, :],
                                start=(dt == 0), stop=(dt == DT - 1),
                            )
                        if fb1sb is not None:
                            nc.scalar.activation(out=hT[:, f, :], in_=ps[:, :],
                                                 func=AF.Relu,
                                                 bias=fb1sb[:, f : f + 1], scale=1.0)
                        else:
                            nc.scalar.activation(out=hT[:, f, :], in_=ps[:, :],
                                                 func=AF.Relu)
            pre = prep.tile([P, DT, W], f32, tag="ffpre")
            acc = lp.tile([P, W], f32, tag="lnacc")
            sqa = lp.tile([P, W], f32, tag="lnsqa")
            FTC = min(8, FT)  # w2 staging chunk (ft tiles per DMA)
            with tc.tile_pool(name="ffps2", bufs=1, space="PSUM") as psq, \
                 tc.tile_pool(name="fflnps", bufs=1, space="PSUM") as lpp:
                for dg in range(DT // G4):
                    ps4 = []
                    for j in range(G4):
                        ps4j = psq.tile([P, W], f32, tag="f2ps%d" % j)
                        ps4.append(ps4j)
                    for fc in range(FT // FTC):
                        w2sl = wp.tile([P, FTC, G4 * P], bf16, tag="f2w")
                        nc.sync.dma_start(
                            out=w2sl[:, :, :],
                            in_=w2T[fc * FTC * P : (fc + 1) * FTC * P,
                                    dg * G4 * P : (dg + 1) * G4 * P]
                            .rearrange("(t p) v -> p t v", p=P),
                        )
                        for fo in range(FTC):
                            ft = fc * FTC + fo
                            for j in range(G4):
                                nc.tensor.matmul(
                                    ps4[j][:, :],
                                    lhsT=w2sl[:, fo, j * P : (j + 1) * P],
                                    rhs=hT[:, ft, :],
                                    start=(ft == 0), stop=(ft == FT - 1),
                                )
                    for j in range(G4):
                        d = dg * G4 + j
                        if fb2sb is not None:
                            tmp = lp.tile([P, W], f32, tag="f2tmp")
                            nc.scalar.activation(out=tmp[:, :], in_=ps4[j][:, :],
                                                 func=AF.Identity,
                                                 bias=fb2sb[:, d : d + 1], scale=1.0)
                            nc.vector.tensor_add(pre[:, d, :], tmp[:, :],
                                                 xf[:, d, :])
                        else:
                            nc.vector.tensor_add(pre[:, d, :], ps4[j][:, :],
                                                 xf[:, d, :])
                        ln_acc_step(pre, d, acc, sqa, lp)
                outp = prep.tile([P, DT, W], f32, tag="ffout")
                ln_t(pre, acc, sqa, None, outp, lns["g3"], lns["b3"], lpp, lp)
                nc.sync.dma_start(
                    out=out.rearrange("(t p) w -> p t w", p=P),
                    in_=outp[:, :, :])


def make_program(cfg, fl):
    from concourse import bacc
    import concourse.tile as tile

    nc = bacc.Bacc("TRN2", target_bir_lowering=False, debug=False,
                   num_devices=8)
    with tile.TileContext(nc) as tc:
        with nc.allow_low_precision(reason="bf16 kernel, rel-err gate 2e-2"):
            _build(nc, tc, cfg, fl)
    nc.compile()
    return nc


def prep_inputs(inputs, cfg):
    """Host-side data prep. Returns (in_maps, fl)."""
    import ml_dtypes

    bf = ml_dtypes.bfloat16
    B, S, D, H, DFF, W, NT, QT = (cfg.B, cfg.S, cfg.D, cfg.H, cfg.DFF,
                                  cfg.W, cfg.NT, cfg.QT)
    f = np.float32
    x = np.asarray(inputs["x"], f)
    enc = np.asarray(inputs["enc_out"])
    trg = np.asarray(inputs["trg_mask"])
    fl = Flags()
    fl.qkb1 = bool(np.any(inputs["qkv_b1"]))
    fl.qkb2 = bool(np.any(inputs["qkv_b2"]))
    fl.vb1 = bool(np.any(np.asarray(inputs["qkv_b1"])[2 * D :]))
    fl.vb2 = bool(np.any(np.asarray(inputs["qkv_b2"])[2 * D :]))
    fl.ob1 = bool(np.any(inputs["out_b1"]))
    fl.ob2 = bool(np.any(inputs["out_b2"]))
    fl.fb1 = bool(np.any(inputs["ff_b1"]))
    fl.fb2 = bool(np.any(inputs["ff_b2"]))
    fl.g1 = not bool(np.all(np.asarray(inputs["ln1_g"]) == 1))
    fl.b1 = bool(np.any(inputs["ln1_b"]))
    fl.g2 = not bool(np.all(np.asarray(inputs["ln2_g"]) == 1))
    fl.b2 = bool(np.any(inputs["ln2_b"]))
    fl.g3 = not bool(np.all(np.asarray(inputs["ln3_g"]) == 1))
    fl.b3 = bool(np.any(inputs["ln3_b"]))
    fl.m1 = not bool(np.all(trg != 0))
    tril = np.tril(np.ones((S, S), np.int32))
    is_tril = (trg.shape[0] == 1 and
               bool(np.array_equal((trg[0, 0] != 0).astype(np.int32), tril)))
    fl.m1full = fl.m1 and not is_tril
    fl.zv2 = bool(np.any(enc == 0))

    def bcast(a):
        return np.ascontiguousarray(np.asarray(a, f).T.astype(bf))

    shared = {
        "qkvwT1": bcast(inputs["qkv_w1"]),
        "qkvwT2": bcast(inputs["qkv_w2"]),
        "owT1": bcast(inputs["out_w1"]),
        "owT2": bcast(inputs["out_w2"]),
        "w1T": bcast(inputs["ff_w1"]),
        "w2T": bcast(inputs["ff_w2"]),
    }
    if fl.qkb1:
        shared["qkvb1"] = np.asarray(inputs["qkv_b1"], f)
    if fl.qkb2:
        shared["qkvb2"] = np.asarray(inputs["qkv_b2"], f)
    if fl.vb1:
        shared["vb1"] = np.broadcast_to(
            np.asarray(inputs["qkv_b1"], f)[2 * D :], (P, D)).astype(bf)
    if fl.vb2:
        shared["vb2"] = np.broadcast_to(
            np.asarray(inputs["qkv_b2"], f)[2 * D :], (P, D)).astype(bf)
    if fl.ob1:
        shared["ob1"] = np.asarray(inputs["out_b1"], f)
    if fl.ob2:
        shared["ob2"] = np.asarray(inputs["out_b2"], f)
    if fl.fb1:
        shared["fb1"] = np.asarray(inputs["ff_b1"], f)
    if fl.fb2:
        shared["fb2"] = np.asarray(inputs["ff_b2"], f)
    for nm, key, use in [("g1", "ln1_g", fl.g1), ("b1", "ln1_b", fl.b1),
                         ("g2", "ln2_g", fl.g2), ("b2", "ln2_b", fl.b2),
                         ("g3", "ln3_g", fl.g3), ("b3", "ln3_b", fl.b3)]:
        if use:
            shared[nm] = np.asarray(inputs[key], f)
    if fl.m1 and not fl.m1full:
        # constant triangular mask for the last QT slots, same on all cores:
        # m1c[k', s, q'] = 1 if s*P + k' <= q'
        kk = np.arange(P)[:, None, None]
        ss = np.arange(QT)[None, :, None]
        qq = np.arange(W)[None, None, :]
        shared["m1c"] = ((ss * P + kk) <= qq).astype(bf)

    xTb = [np.ascontiguousarray(x[b].T).astype(bf) for b in range(B)]
    in_maps = []
    for c in range(8):
        b, r = c // 4, c % 4
        m = dict(shared)
        # rotate key tiles: slot t holds physical tile p(t) = (t+(r+1)*QT)%NT
        perm = [(t + (r + 1) * QT) % NT for t in range(NT)]
        xt = xTb[b].reshape(D, NT, P)
        m["xTr"] = np.ascontiguousarray(
            xt[:, perm, :].reshape(D, S))
        if fl.m1 and not fl.m1full:
            zv = np.zeros((P, NT), f)
            for t in range(NT):
                if perm[t] < (r + 1) * QT:
                    zv[:, t] = 1
            m["zv1"] = zv
        if fl.m1full:
            # generic multiplicative mask in rotated key coordinates
            tb = trg[b] if trg.shape[0] == B else trg[0]
            blk = (tb[0, r * W : (r + 1) * W, :] != 0).astype(f)  # [W, S](q,k)
            mk = blk.T.reshape(NT, P, W)  # [kt, k', q]
            m["m1f"] = np.ascontiguousarray(mk[perm]).astype(bf)
        if fl.zv2:
            # this core's own quarter only (keys it contributes to the
            # gathered V2) -- applied before the gather, so consumers see
            # already-zeroed rows
            eb = (np.asarray(enc)[b, 0, 0, r * W : (r + 1) * W] != 0).astype(f)
            m["zv2"] = np.ascontiguousarray(
                eb.reshape(W // P, P).T).astype(f)
        in_maps.append(m)
    return in_maps, fl


def kernel_with_results(**inputs):
    from concourse.bass_utils import run_bass_kernel_spmd

    cfg = Cfg()
    x = np.asarray(inputs["x"])
    assert x.shape == (cfg.B, cfg.S, cfg.D), x.shape
    in_maps, fl = prep_inputs(inputs, cfg)
    nc = make_program(cfg, fl)
    res = run_bass_kernel_spmd(nc, in_maps, list(range(8)))
    y = np.empty((cfg.B, cfg.S, cfg.D), np.float32)
    for c in range(8):
        b, r = c // 4, c % 4
        y[b, r * cfg.W : (r + 1) * cfg.W, :] = res.results[c]["out"].T
    return y, res


def kernel(**inputs):
    return kernel_with_results(**inputs)[0]


# revision 35
# speedup vs baseline: 1.4282x; 1.0555x over previous
"""Trainium2 Bass kernel for nn_DecoderLayer (dense transformer decoder layer).

Strategy (8 NeuronCores, full inputs in / full output out):
  - core c handles batch b = c//4 and query-quarter r = c%4 (rows [r*S/4, (r+1)*S/4)).
  - All matmul operands are bf16 (PSUM accumulation fp32); activations are
    kept TRANSPOSED on-chip (x^T [D, n]) so projections run with the
    contraction dim on partitions.
  - K^T / V / Q^T live entirely in SBUF (no HBM round-trip).
  - Self-attention causal masking: the host rotates each core's key order so
    the 4 "diagonal" key tiles sit at fixed slots (last group); keys fully
    below the diagonal get their V rows zeroed at projection time (per-key
    0/1 scale fused into the PSUM evacuation), so only the last exp group
    needs a (core-independent, constant) triangular multiplicative mask.
  - Cross-attention key masking (enc mask) uses the same V-row zeroing.
  - Softmax denominators come free from a ones column appended to V.
  - The single collective: AllGather of x1 (post-LN1) within each 4-core
    batch group, needed because cross-attention K2/V2 are projections of the
    full x1.
  - LayerNorm runs in transposed layout: cross-partition sums via ones-matmul
    on the PE, stats broadcast back to [128, W] via ones-matmul.
"""

import sys

if "/opt/trn_rl_repo" not in sys.path:
    sys.path.insert(0, "/opt/trn_rl_repo")

import numpy as np

P = 128
HD = 64
HD1 = HD + 1
EPS = 1e-5


class Cfg:
    def __init__(self, B=2, S=2048, D=1024, H=16, DFF=4096, use_collective=True,
                 fake_gather=False):
        self.B, self.S, self.D, self.H, self.DFF = B, S, D, H, DFF
        self.fake_gather = fake_gather
        self.W = S // 4            # local query rows per core
        self.DT = D // P           # feature-dim tiles
        self.NT = S // P           # sequence tiles (keys)
        self.FT = DFF // P         # ffn hidden tiles
        self.HP = P // HD          # heads per partition-tile (2)
        self.QT = self.W // P      # key tiles per query quarter (diag tiles)
        self.KTG = min(2, self.NT)    # k-tiles per exp group
        self.NG = self.NT // self.KTG
        self.VCW = min(512, D)        # v-dout chunk width
        self.VCN = D // self.VCW
        self.HPC = self.VCW // HD     # heads per v-chunk
        self.KCW = min(512, self.S)   # k-proj token chunk width
        self.KCN = self.S // self.KCW
        self.G4 = min(4, self.DT)
        self.use_collective = use_collective
        assert D == H * HD
        assert self.W % P == 0 and D % P == 0 and DFF % P == 0 and S % P == 0


class Flags:
    def __init__(self):
        self.qkb1 = self.vb1 = self.ob1 = False
        self.qkb2 = self.vb2 = self.ob2 = False
        self.fb1 = self.fb2 = False
        self.g1 = self.b1 = self.g2 = self.b2 = self.g3 = self.b3 = False
        self.m1 = True      # trg mask active (tril fast path)
        self.m1full = False  # generic (non-tril) trg mask: full mult tiles
        self.zv2 = False    # enc mask active -> zero V2 rows


def _build(nc, tc, cfg, fl):
    import concourse.bass as bass
    import concourse.mybir as mybir
    import concourse.tile as tile  # noqa: F401
    from contextlib import ExitStack

    AF = mybir.ActivationFunctionType
    f32 = mybir.dt.float32
    bf16 = mybir.dt.bfloat16

    B, S, D, H, DFF = cfg.B, cfg.S, cfg.D, cfg.H, cfg.DFF
    W, DT, NT, FT, HP = cfg.W, cfg.DT, cfg.NT, cfg.FT, cfg.HP
    QT, KTG, NG = cfg.QT, cfg.KTG, cfg.NG
    VCW, VCN, HPC = cfg.VCW, cfg.VCN, cfg.HPC
    KCW, KCN, G4 = cfg.KCW, cfg.KCN, cfg.G4

    # ---------------- DRAM parameters ----------------
    def din(name, shape, dt=bf16):
        return nc.dram_tensor(name, shape, dt, kind="ExternalInput").ap()

    xTr = din("xTr", [D, S])          # rotated x^T for this core
    qkvwT1 = din("qkvwT1", [D, 3 * D])
    qkvwT2 = din("qkvwT2", [D, 3 * D])
    owT1 = din("owT1", [D, D])
    owT2 = din("owT2", [D, D])
    w1T = din("w1T", [D, DFF])
    w2T = din("w2T", [DFF, D])
    m1c = din("m1c", [P, QT, W]) if (fl.m1 and not fl.m1full) else None
    m1f = din("m1f", [NT, P, W]) if fl.m1full else None
    zv1 = din("zv1", [P, NT], f32) if fl.m1 and not fl.m1full else None
    zv2d = din("zv2", [P, QT], f32) if fl.zv2 else None
    qkvb1 = din("qkvb1", [3 * D], f32) if fl.qkb1 else None
    qkvb2 = din("qkvb2", [3 * D], f32) if fl.qkb2 else None
    vb1 = din("vb1", [P, D]) if fl.vb1 else None
    vb2 = din("vb2", [P, D]) if fl.vb2 else None
    ob1 = din("ob1", [D], f32) if fl.ob1 else None
    ob2 = din("ob2", [D], f32) if fl.ob2 else None
    fb1d = din("fb1", [DFF], f32) if fl.fb1 else None
    fb2d = din("fb2", [D], f32) if fl.fb2 else None
    lnp = {}
    for nm, use in [("g1", fl.g1), ("b1", fl.b1), ("g2", fl.g2),
                    ("b2", fl.b2), ("g3", fl.g3), ("b3", fl.b3)]:
        lnp[nm] = din(nm, [D], f32) if use else None
    out = nc.dram_tensor("out", [D, W], f32, kind="ExternalOutput").ap()

    es = ExitStack()
    with es:
        dramp = es.enter_context(tc.tile_pool(name="dram", bufs=1, space="DRAM"))
        LKV = D * W + W * H * HD1  # flat K2loc + V2loc staging elements
        if cfg.use_collective:
            kvs = dramp.tile([LKV], bf16)
            agkv = dramp.tile([4 * LKV], bf16)

        const = es.enter_context(tc.tile_pool(name="const", bufs=1))
        ones_p1 = const.tile([P, 1], f32)
        nc.vector.memset(ones_p1[:, :], 1.0)
        ones_1p = const.tile([1, P], f32)
        nc.vector.memset(ones_1p[0:1, :], 1.0)
        ones_col = const.tile([P, HPC, 1], bf16)
        nc.vector.memset(ones_col[:, :, :], 1.0)
        eps_t = const.tile([1, 1], f32)
        nc.vector.memset(eps_t[0:1, :], EPS)

        def ldvec(dram_vec, n_tiles, name):
            """[D]-style f32 vector -> [P, n_tiles] sbuf tile."""
            t = const.tile([P, n_tiles], f32, tag=name)
            nc.sync.dma_start(
                out=t[:, :],
                in_=dram_vec.rearrange("(t p) -> p t", p=P),
            )
            return t

        qkb1sb = ldvec(qkvb1[0 : 2 * D], 2 * DT, "qkb1") if fl.qkb1 else None
        qkb2sb = ldvec(qkvb2[0 : 2 * D], 2 * DT, "qkb2") if fl.qkb2 else None
        ob1sb = ldvec(ob1, DT, "ob1") if fl.ob1 else None
        ob2sb = ldvec(ob2, DT, "ob2") if fl.ob2 else None
        fb1sb = ldvec(fb1d, FT, "fb1") if fl.fb1 else None
        fb2sb = ldvec(fb2d, DT, "fb2") if fl.fb2 else None
        lns = {k: (ldvec(v, DT, "ln" + k) if v is not None else None)
               for k, v in lnp.items()}
        vb1sb = None
        if fl.vb1:
            vb1sb = const.tile([P, D], bf16, tag="vb1")
            nc.sync.dma_start(out=vb1sb[:, :], in_=vb1[:, :])
        vb2sb = None
        if fl.vb2:
            vb2sb = const.tile([P, D], bf16, tag="vb2")
            nc.sync.dma_start(out=vb2sb[:, :], in_=vb2[:, :])
        zv1sb = None
        if zv1 is not None:
            zv1sb = const.tile([P, NT], f32, tag="zv1")
            nc.sync.dma_start(out=zv1sb[:, :], in_=zv1[:, :])
        zv2sb = None
        if zv2d is not None:
            zv2sb = const.tile([P, QT], f32, tag="zv2")
            nc.sync.dma_start(out=zv2sb[:, :], in_=zv2d[:, :])
        m1sb = None
        if m1c is not None:
            m1sb = const.tile([P, QT, W], bf16, tag="m1c")
            nc.sync.dma_start(out=m1sb[:, :, :], in_=m1c[:, :, :])
        m1fsb = None
        if m1f is not None:
            m1fsb = const.tile([P, NT, W], bf16, tag="m1f")
            nc.sync.dma_start(out=m1fsb[:, :, :], in_=m1f.rearrange("n p w -> p n w"))

        # persistent mid tensors
        midp = es.enter_context(tc.tile_pool(name="mid", bufs=1))
        qT = midp.tile([P, DT, W], bf16)      # Q^T local (reused block2)
        aoT = midp.tile([P, DT, W], bf16)     # attention out^T (reused)
        x1T = midp.tile([P, DT, W], bf16)     # x1 local
        x2T = midp.tile([P, DT, W], bf16)     # x2 local
        xlocT = midp.tile([P, DT, W], bf16)   # this core's x quarter (resid 1)
        xf = midp.tile([P, DT, W], f32)       # f32 residual backbone (x1, x2)

        # =========== QKV projection (into SBUF K/V/Q) ===========
        def qkv_phase(xsb, xq, wT, kT, v, qkb, vbsb, zvsb):
            """xsb: [P, DT, Sx] bf16 x^T source (tokens = key order);
            xq: [P, DT, W] bf16 x^T source for this core's queries;
            writes kT [P, DT, Sx] sbuf, v [P, NT_x, H, HD1] sbuf, qT."""
            Sx = xsb.shape[2]
            NTx = Sx // P
            KCWx = min(KCW, Sx)
            KCNx = Sx // KCWx
            with tc.tile_pool(name="qkv_w", bufs=2) as wp, \
                 tc.tile_pool(name="qkv_ps", bufs=2, space="PSUM") as psp:
                # ---- K^T ----
                wall = wp.tile([P, DT, D], bf16, tag="wall")
                nc.sync.dma_start(
                    out=wall[:, :, :],
                    in_=wT[:, D : 2 * D].rearrange("(t p) v -> p t v", p=P),
                )
                for nch in range(KCNx):
                    for dk in range(DT):
                        ps = psp.tile([P, KCWx], f32, tag="kps")
                        for dt in range(DT):
                            nc.tensor.matmul(
                                ps[:, :],
                                lhsT=wall[:, dt, dk * P : (dk + 1) * P],
                                rhs=xsb[:, dt, nch * KCWx : (nch + 1) * KCWx],
                                start=(dt == 0),
                                stop=(dt == DT - 1),
                            )
                        if qkb is not None:
                            nc.scalar.activation(
                                out=kT[:, dk, nch * KCWx : (nch + 1) * KCWx],
                                in_=ps[:, :], func=AF.Identity,
                                bias=qkb[:, DT + dk : DT + dk + 1], scale=1.0,
                            )
                        else:
                            nc.scalar.activation(
                                out=kT[:, dk, nch * KCWx : (nch + 1) * KCWx],
                                in_=ps[:, :], func=AF.Copy,
                            )
                # ---- Q^T local [D, W] ----
                wall = wp.tile([P, DT, D], bf16, tag="wall")
                nc.sync.dma_start(
                    out=wall[:, :, :],
                    in_=wT[:, 0:D].rearrange("(t p) v -> p t v", p=P),
                )
                for dq in range(DT):
                    ps = psp.tile([P, W], f32, tag="qps")
                    for dt in range(DT):
                        nc.tensor.matmul(
                            ps[:, :],
                            lhsT=wall[:, dt, dq * P : (dq + 1) * P],
                            rhs=xq[:, dt, :],
                            start=(dt == 0),
                            stop=(dt == DT - 1),
                        )
                    if qkb is not None:
                        nc.scalar.activation(
                            out=qT[:, dq, :], in_=ps[:, :], func=AF.Identity,
                            bias=qkb[:, dq : dq + 1], scale=1.0,
                        )
                    else:
                        nc.scalar.activation(
                            out=qT[:, dq, :], in_=ps[:, :], func=AF.Copy,
                        )
                # ---- V natural [n, dout] + ones column ----
                wall = wp.tile([P, DT, D], bf16, tag="wall")
                nc.sync.dma_start(
                    out=wall[:, :, :],
                    in_=wT[:, 2 * D : 3 * D].rearrange("(t p) v -> p t v", p=P),
                )
                for nt in range(NTx):
                    for vc in range(VCN):
                        ps = psp.tile([P, VCW], f32, tag="vps")
                        for dt in range(DT):
                            nc.tensor.matmul(
                                ps[:, :],
                                lhsT=xsb[:, dt, nt * P : (nt + 1) * P],
                                rhs=wall[:, dt, vc * VCW : (vc + 1) * VCW],
                                start=(dt == 0),
                                stop=(dt == DT - 1),
                            )
                        dst = v[:, nt, vc * HPC : (vc + 1) * HPC, 0:HD]
                        psv = ps.rearrange("p (h d) -> p h d", d=HD)
                        if vbsb is not None:
                            # bias first, then per-key zeroing (mask applies
                            # to the biased value)
                            nc.scalar.activation(out=dst, in_=psv, func=AF.Copy)
                            nc.vector.tensor_add(
                                dst, dst,
                                vbsb[:, vc * VCW : (vc + 1) * VCW].rearrange(
                                    "p (h d) -> p h d", d=HD),
                            )
                            if zvsb is not None:
                                nc.vector.tensor_scalar_mul(
                                    dst, dst, zvsb[:, nt : nt + 1])
                        elif zvsb is not None:
                            nc.scalar.activation(
                                out=dst, in_=psv, func=AF.Copy,
                                scale=zvsb[:, nt : nt + 1],
                            )
                        else:
                            nc.scalar.activation(out=dst, in_=psv, func=AF.Copy)
                        oc = v[:, nt, vc * HPC : (vc + 1) * HPC, HD:HD1]
                        if zvsb is not None:
                            nc.scalar.activation(
                                out=oc, in_=ones_col[:, :, :], func=AF.Copy,
                                scale=zvsb[:, nt : nt + 1],
                            )
                        else:
                            nc.vector.memset(oc, 1.0)

        # =========== attention phase ===========
        def attn_phase(kT, v, msb, mfull, warm=False):
            """msb: [P, QT, W] triangular mask on the LAST QT slots (or None);
            mfull: [P, NT, W] generic multiplicative mask (or None).
            Score PSUM is manually double-buffered (two KTG-slot halves of one
            [P, 2*KTG, W] tile) so the next group's matmuls never wait on the
            previous group's exp."""
            NTx = v.shape[1]
            NGx = NTx // KTG
            with tc.tile_pool(name="at_ex", bufs=3) as exp_, \
                 tc.tile_pool(name="at_dn", bufs=2) as dnp, \
                 tc.tile_pool(name="at_ps", bufs=2, space="PSUM") as psp, \
                 tc.tile_pool(name="at_po", bufs=2, space="PSUM") as pop, \
                 tc.tile_pool(name="at_wm", bufs=1, space="PSUM") as wmp:
                # attention matmuls use at most half the PE array (64-row
                # contraction for scores, 65-col output for AV), which the
                # HAM activity monitor does not count as busy -- so the PE
                # clock stays at whatever state attention entered with. A
                # full-array 128x128 matmul interleaved every other group
                # keeps tripping the monitor so the array runs at 2.4GHz.
                wps = wmp.tile([P, W], f32, tag="warm")
                for h in range(H):
                    hh = (h % HP) * HD
                    q_h = qT[hh : hh + HD, h // HP, :]
                    po = pop.tile([P, W], f32, tag="po")
                    pstiles = {}

                    def scores(g):
                        ps = psp.tile([P, KTG, W], f32, tag="sc")
                        pstiles[g] = ps
                        for o in range(KTG):
                            kt = g * KTG + o
                            nc.tensor.matmul(
                                ps[:, o, :],
                                lhsT=kT[hh : hh + HD, h // HP,
                                        kt * P : (kt + 1) * P],
                                rhs=q_h,
                                start=True,
                                stop=True,
                            )

                    # software-pipelined: scores(g+1) is issued to the PE
                    # ahead of AV(g), so the PE never sits behind an AV that
                    # is itself waiting on exp(g).
                    scores(0)
                    for g in range(NGx):
                        if g + 1 < NGx:
                            scores(g + 1)
                        if warm and g % 2 == 0:
                            nc.tensor.matmul(
                                wps[:, :], lhsT=kT[:, 0, 0:P],
                                rhs=kT[:, 0, 0:W], start=True, stop=True,
                            )
                        ps = pstiles.pop(g)
                        ex = exp_.tile([P, KTG, W], bf16, tag="ex")
                        nc.scalar.activation(
                            out=ex[:, :, :], in_=ps[:, :, :], func=AF.Exp,
                            scale=1.0 / float(np.sqrt(HD)),
                        )
                        if mfull is not None:
                            nc.vector.tensor_mul(
                                ex[:, :, :], ex[:, :, :],
                                mfull[:, g * KTG : (g + 1) * KTG, :],
                            )
                        elif msb is not None:
                            # overlap of this group's slots with the diagonal
                            # region [NTx-QT, NTx)
                            lo = max(g * KTG, NTx - QT)
                            hi = (g + 1) * KTG
                            if lo < hi:
                                nc.vector.tensor_mul(
                                    ex[:, lo - g * KTG : KTG, :],
                                    ex[:, lo - g * KTG : KTG, :],
                                    msb[:, lo - (NTx - QT) : hi - (NTx - QT), :],
                                )
                        for o in range(KTG):
                            kt = g * KTG + o
                            nc.tensor.matmul(
                                po[0:HD1, :],
                                lhsT=v[:, kt, h, :],
                                rhs=ex[:, o, :],
                                start=(g == 0 and o == 0),
                                stop=(g == NGx - 1 and o == KTG - 1),
                            )
                    dinv = dnp.tile([1, W], f32, tag="dinv")
                    nc.vector.reciprocal(dinv[0:1, :], po[HD:HD1, :])
                    dinvb = dnp.tile([HD, W], f32, tag="dinvb")
                    nc.gpsimd.partition_broadcast(
                        dinvb[0:HD, :], dinv[0:1, :], channels=HD
                    )
                    nc.vector.tensor_mul(
                        aoT[hh : hh + HD, h // HP, :],
                        po[0:HD, :],
                        dinvb[0:HD, :],
                    )

        # =========== layernorm (transposed layout, f32 internals) ===========
        def ln_t(pre, acc, sqa, out_bf, out_f, g_sb, b_sb, lpp, lp):
            """pre: [P, DT, W] f32 sbuf; acc/sqa already accumulated via
            ln_acc_step; out_bf bf16 (or None), out_f f32 (or None)."""
            sums = lpp.tile([1, W], f32, tag="lnsums")
            nc.tensor.matmul(sums[0:1, :], lhsT=ones_p1[:, :],
                             rhs=acc[:, :], start=True, stop=True)
            sqs = lpp.tile([1, W], f32, tag="lnsqs")
            nc.tensor.matmul(sqs[0:1, :], lhsT=ones_p1[:, :],
                             rhs=sqa[:, :], start=True, stop=True)
            mu = lp.tile([1, W], f32, tag="lnmu")
            nc.vector.tensor_scalar_mul(mu[0:1, :], sums[0:1, :], 1.0 / D)
            ex2 = lp.tile([1, W], f32, tag="lnex2")
            nc.vector.tensor_scalar_mul(ex2[0:1, :], sqs[0:1, :], 1.0 / D)
            mu2 = lp.tile([1, W], f32, tag="lnmu2")
            nc.scalar.square(mu2[0:1, :], mu[0:1, :])
            var = lp.tile([1, W], f32, tag="lnvar")
            nc.vector.tensor_sub(var[0:1, :], ex2[0:1, :], mu2[0:1, :])
            sd = lp.tile([1, W], f32, tag="lnsd")
            nc.scalar.activation(out=sd[0:1, :], in_=var[0:1, :], func=AF.Sqrt,
                                 bias=eps_t[0:1, :], scale=1.0)
            rstd = lp.tile([1, W], f32, tag="lnrstd")
            nc.vector.reciprocal(rstd[0:1, :], sd[0:1, :])
            mub = lpp.tile([P, W], f32, tag="lnmub")
            nc.tensor.matmul(mub[:, :], lhsT=ones_1p[0:1, :],
                             rhs=mu[0:1, :], start=True, stop=True)
            rstdb = lpp.tile([P, W], f32, tag="lnrstdb")
            nc.tensor.matmul(rstdb[:, :], lhsT=ones_1p[0:1, :],
                             rhs=rstd[0:1, :], start=True, stop=True)
            mubs = lp.tile([P, W], f32, tag="lnmubs")
            nc.vector.tensor_copy(mubs[:, :], mub[:, :])
            rstdbs = lp.tile([P, W], f32, tag="lnrstdbs")
            nc.vector.tensor_copy(rstdbs[:, :], rstdb[:, :])
            for d in range(DT):
                t1 = lp.tile([P, W], f32, tag="lnt1")
                nc.vector.tensor_sub(t1[:, :], pre[:, d, :], mubs[:, :])
                of = out_f[:, d, :] if out_f is not None else None
                if of is not None:
                    nc.vector.tensor_mul(of, t1[:, :], rstdbs[:, :])
                    if g_sb is not None:
                        nc.vector.tensor_scalar_mul(of, of, g_sb[:, d : d + 1])
                    if b_sb is not None:
                        nc.vector.tensor_scalar_add(of, of, b_sb[:, d : d + 1])
                    if out_bf is not None:
                        nc.vector.tensor_copy(out_bf[:, d, :], of)
                else:
                    ob = out_bf[:, d, :]
                    nc.vector.tensor_mul(ob, t1[:, :], rstdbs[:, :])
                    if g_sb is not None:
                        nc.vector.tensor_scalar_mul(ob, ob, g_sb[:, d : d + 1])
                    if b_sb is not None:
                        nc.vector.tensor_scalar_add(ob, ob, b_sb[:, d : d + 1])

        # ---- incremental LN stat accumulation (overlaps producer loops) ----
        def ln_acc_step(pre, d, acc, sqa, lp):
            if d == 0:
                nc.vector.tensor_copy(acc[:, :], pre[:, 0, :])
                nc.scalar.square(sqa[:, :], pre[:, 0, :])
            else:
                nc.vector.tensor_add(acc[:, :], acc[:, :], pre[:, d, :])
                sqt = lp.tile([P, W], f32, tag="lnsqt")
                nc.scalar.square(sqt[:, :], pre[:, d, :])
                nc.vector.tensor_add(sqa[:, :], sqa[:, :], sqt[:, :])

        # =========== out-projection + residual + LN ===========
        def proj_resid_ln(owT, obsb, residT, g_sb, b_sb, out_bf, out_f):
            with tc.tile_pool(name="pr_w", bufs=2) as wp, \
                 tc.tile_pool(name="pr_t", bufs=2) as lp, \
                 tc.tile_pool(name="pr_pre", bufs=1) as prep, \
                 tc.tile_pool(name="pr_ps", bufs=2, space="PSUM") as psp, \
                 tc.tile_pool(name="pr_lnps", bufs=1, space="PSUM") as lpp:
                pre = prep.tile([P, DT, W], f32, tag="pre")
                acc = lp.tile([P, W], f32, tag="lnacc")
                sqa = lp.tile([P, W], f32, tag="lnsqa")
                for dg in range(DT // G4):
                    wsl = wp.tile([P, DT, G4 * P], bf16, tag="prw")
                    nc.sync.dma_start(
                        out=wsl[:, :, :],
                        in_=owT[:, dg * G4 * P : (dg + 1) * G4 * P]
                        .rearrange("(t p) v -> p t v", p=P),
                    )
                    for j in range(G4):
                        d = dg * G4 + j
                        ps = psp.tile([P, W], f32, tag="prps")
                        for dt in range(DT):
                            nc.tensor.matmul(
                                ps[:, :], lhsT=wsl[:, dt, j * P : (j + 1) * P],
                                rhs=aoT[:, dt, :],
                                start=(dt == 0), stop=(dt == DT - 1),
                            )
                        if obsb is not None:
                            tmp = lp.tile([P, W], f32, tag="prtmp")
                            nc.scalar.activation(out=tmp[:, :], in_=ps[:, :],
                                                 func=AF.Identity,
                                                 bias=obsb[:, d : d + 1], scale=1.0)
                            nc.vector.tensor_add(pre[:, d, :], tmp[:, :],
                                                 residT[:, d, :])
                        else:
                            nc.vector.tensor_add(pre[:, d, :], ps[:, :],
                                                 residT[:, d, :])
                        ln_acc_step(pre, d, acc, sqa, lp)
                ln_t(pre, acc, sqa, out_bf, out_f, g_sb, b_sb, lpp, lp)

        # ================= block 1: self-attention =================
        with tc.tile_pool(name="kv1", bufs=1) as kvp1:
            kT1 = kvp1.tile([P, DT, S], bf16)
            v1 = kvp1.tile([P, NT, H, HD1], bf16)
            with tc.tile_pool(name="xs1", bufs=1) as xsp1:
                xs = xsp1.tile([P, DT, S], bf16)
                nc.sync.dma_start(out=xs[:, :, :],
                                  in_=xTr.rearrange("(t p) s -> p t s", p=P))
                # the host always rotates key order so this core's quarter
                # sits in the last QT slots (uniform across cores); any mask
                # data is supplied in rotated coordinates.
                xq_off = S - W
                xloc = xs[:, :, xq_off : xq_off + W]
                nc.vector.tensor_copy(xlocT[:, :, :], xloc)
                qkv_phase(xs, xloc, qkvwT1, kT1, v1, qkb1sb, vb1sb, zv1sb)
            attn_phase(kT1, v1, m1sb, m1fsb)
        proj_resid_ln(owT1, ob1sb, xlocT, lns["g1"], lns["b1"], x1T, xf)

        # ---- local K2/V2/Q2 from x1, then all-gather K2|V2 in group ----
        assert cfg.use_collective
        with tc.tile_pool(name="kvloc", bufs=1) as kvlp:
            klocT = kvlp.tile([P, DT, W], bf16)
            vloc = kvlp.tile([P, QT, H, HD1], bf16)
            qkv_phase(x1T, x1T, qkvwT2, klocT, vloc, qkb2sb, vb2sb, zv2sb)
            nc.sync.dma_start(
                out=kvs[0 : D * W].rearrange("(t p w) -> p t w", p=P, w=W),
                in_=klocT[:, :, :],
            )
            nc.sync.dma_start(
                out=kvs[D * W : LKV].rearrange("(q p h d) -> p q h d",
                                               p=P, h=H, d=HD1),
                in_=vloc[:, :, :, :],
            )
        if cfg.fake_gather:
            for g in range(4):
                nc.sync.dma_start(out=agkv[g * LKV : (g + 1) * LKV],
                                  in_=kvs[:])
        else:
            nc.gpsimd.collective_compute(
                "AllGather",
                bass.mybir.AluOpType.bypass,
                replica_groups=[[0, 1, 2, 3], [4, 5, 6, 7]],
                ins=[kvs[:]],
                outs=[agkv[:]],
            )



        # ================= block 2: cross-attention =================
        with tc.tile_pool(name="kv2", bufs=1) as kvp2:
            kT2 = kvp2.tile([P, DT, S], bf16)
            v2 = kvp2.tile([P, NT, H, HD1], bf16)
            for g in range(4):
                nc.sync.dma_start(
                    out=kT2[:, :, g * W : (g + 1) * W],
                    in_=agkv[g * LKV : g * LKV + D * W]
                    .rearrange("(t p w) -> p t w", p=P, w=W),
                )
                nc.sync.dma_start(
                    out=v2[:, g * QT : (g + 1) * QT, :, :],
                    in_=agkv[g * LKV + D * W : (g + 1) * LKV]
                    .rearrange("(q p h d) -> p q h d", p=P, h=H, d=HD1),
                )
            attn_phase(kT2, v2, None, None, warm=True)
        proj_resid_ln(owT2, ob2sb, xf, lns["g2"], lns["b2"], x2T, xf)

        # ================= FFN =================
        with tc.tile_pool(name="ffh", bufs=1) as fhp, \
             tc.tile_pool(name="ffw", bufs=2) as wp, \
             tc.tile_pool(name="fft", bufs=1) as lp, \
             tc.tile_pool(name="ffpre", bufs=1) as prep:
            hT = fhp.tile([P, FT, W], bf16)
            with tc.tile_pool(name="ffps1", bufs=2, space="PSUM") as psp:
                for fg in range(FT // G4):
                    wsl = wp.tile([P, DT, G4 * P], bf16, tag="f1w")
                    nc.sync.dma_start(
                        out=wsl[:, :, :],
                        in_=w1T[:, fg * G4 * P : (fg + 1) * G4 * P]
                        .rearrange("(t p) v -> p t v", p=P),
                    )
                    for j in range(G4):
                        f = fg * G4 + j
                        ps = psp.tile([P, W], f32, tag="f1ps")
                        for dt in range(DT):
                            nc.tensor.matmul(
                                ps[:, :], lhsT=wsl[:, dt, j * P : (j + 1) * P],
                                rhs=x2T[:, dt, :],
                                start=(dt == 0), stop=(dt == DT - 1),
                            )
                        if fb1sb is not None:
                            nc.scalar.activation(out=hT[:, f, :], in_=ps[:, :],
                                                 func=AF.Relu,
                                                 bias=fb1sb[:, f : f + 1], scale=1.0)
                        else:
                            nc.scalar.activation(out=hT[:, f, :], in_=ps[:, :],
                                                 func=AF.Relu)
            pre = prep.tile([P, DT, W], f32, tag="ffpre")
            acc = lp.tile([P, W], f32, tag="lnacc")
            sqa = lp.tile([P, W], f32, tag="lnsqa")
            FTC = min(8, FT)  # w2 staging chunk (ft tiles per DMA)
            with tc.tile_pool(name="ffps2", bufs=1, space="PSUM") as psq, \
                 tc.tile_pool(name="fflnps", bufs=1, space="PSUM") as lpp:
                for dg in range(DT // G4):
                    ps4 = []
                    for j in range(G4):
                        ps4j = psq.tile([P, W], f32, tag="f2ps%d" % j)
                        ps4.append(ps4j)
                    for fc in range(FT // FTC):
                        w2sl = wp.tile([P, FTC, G4 * P], bf16, tag="f2w")
                        nc.sync.dma_start(
                            out=w2sl[:, :, :],
                            in_=w2T[fc * FTC * P : (fc + 1) * FTC * P,
                                    dg * G4 * P : (dg + 1) * G4 * P]
                            .rearrange("(t p) v -> p t v", p=P),
                        )
                        for fo in range(FTC):
                            ft = fc * FTC + fo
                            for j in range(G4):
                                nc.tensor.matmul(
                                    ps4[j][:, :],
                                    lhsT=w2sl[:, fo, j * P : (j + 1) * P],
                                    rhs=hT[:, ft, :],
                                    start=(ft == 0), stop=(ft == FT - 1),
                                )
                    for j in range(G4):
                        d = dg * G4 + j
                        if fb2sb is not None:
                            tmp = lp.tile([P, W], f32, tag="f2tmp")
                            nc.scalar.activation(out=tmp[:, :], in_=ps4[j][:, :],
                                                 func=AF.Identity,
                                                 bias=fb2sb[:, d : d + 1], scale=1.0)
                            nc.vector.tensor_add(pre[:, d, :], tmp[:, :],
                                                 xf[:, d, :])
                        else:
                            nc.vector.tensor_add(pre[:, d, :], ps4[j][:, :],
                                                 xf[:, d, :])
                        ln_acc_step(pre, d, acc, sqa, lp)
                outp = prep.tile([P, DT, W], f32, tag="ffout")
                ln_t(pre, acc, sqa, None, outp, lns["g3"], lns["b3"], lpp, lp)
                nc.sync.dma_start(
                    out=out.rearrange("(t p) w -> p t w", p=P),
                    in_=outp[:, :, :])


def make_program(cfg, fl):
    from concourse import bacc
    import concourse.tile as tile

    nc = bacc.Bacc("TRN2", target_bir_lowering=False, debug=False,
                   num_devices=8)
    with tile.TileContext(nc) as tc:
        with nc.allow_low_precision(reason="bf16 kernel, rel-err gate 2e-2"):
            _build(nc, tc, cfg, fl)
    nc.compile()
    return nc


def prep_inputs(inputs, cfg):
    """Host-side data prep. Returns (in_maps, fl)."""
    import ml_dtypes

    bf = ml_dtypes.bfloat16
    B, S, D, H, DFF, W, NT, QT = (cfg.B, cfg.S, cfg.D, cfg.H, cfg.DFF,
                                  cfg.W, cfg.NT, cfg.QT)
    f = np.float32
    x = np.asarray(inputs["x"], f)
    enc = np.asarray(inputs["enc_out"])
    trg = np.asarray(inputs["trg_mask"])
    fl = Flags()
    fl.qkb1 = bool(np.any(inputs["qkv_b1"]))
    fl.qkb2 = bool(np.any(inputs["qkv_b2"]))
    fl.vb1 = bool(np.any(np.asarray(inputs["qkv_b1"])[2 * D :]))
    fl.vb2 = bool(np.any(np.asarray(inputs["qkv_b2"])[2 * D :]))
    fl.ob1 = bool(np.any(inputs["out_b1"]))
    fl.ob2 = bool(np.any(inputs["out_b2"]))
    fl.fb1 = bool(np.any(inputs["ff_b1"]))
    fl.fb2 = bool(np.any(inputs["ff_b2"]))
    fl.g1 = not bool(np.all(np.asarray(inputs["ln1_g"]) == 1))
    fl.b1 = bool(np.any(inputs["ln1_b"]))
    fl.g2 = not bool(np.all(np.asarray(inputs["ln2_g"]) == 1))
    fl.b2 = bool(np.any(inputs["ln2_b"]))
    fl.g3 = not bool(np.all(np.asarray(inputs["ln3_g"]) == 1))
    fl.b3 = bool(np.any(inputs["ln3_b"]))
    fl.m1 = not bool(np.all(trg != 0))
    tril = np.tril(np.ones((S, S), np.int32))
    is_tril = (trg.shape[0] == 1 and
               bool(np.array_equal((trg[0, 0] != 0).astype(np.int32), tril)))
    fl.m1full = fl.m1 and not is_tril
    fl.zv2 = bool(np.any(enc == 0))

    def bcast(a):
        return np.ascontiguousarray(np.asarray(a, f).T.astype(bf))

    shared = {
        "qkvwT1": bcast(inputs["qkv_w1"]),
        "qkvwT2": bcast(inputs["qkv_w2"]),
        "owT1": bcast(inputs["out_w1"]),
        "owT2": bcast(inputs["out_w2"]),
        "w1T": bcast(inputs["ff_w1"]),
        "w2T": bcast(inputs["ff_w2"]),
    }
    if fl.qkb1:
        shared["qkvb1"] = np.asarray(inputs["qkv_b1"], f)
    if fl.qkb2:
        shared["qkvb2"] = np.asarray(inputs["qkv_b2"], f)
    if fl.vb1:
        shared["vb1"] = np.broadcast_to(
            np.asarray(inputs["qkv_b1"], f)[2 * D :], (P, D)).astype(bf)
    if fl.vb2:
        shared["vb2"] = np.broadcast_to(
            np.asarray(inputs["qkv_b2"], f)[2 * D :], (P, D)).astype(bf)
    if fl.ob1:
        shared["ob1"] = np.asarray(inputs["out_b1"], f)
    if fl.ob2:
        shared["ob2"] = np.asarray(inputs["out_b2"], f)
    if fl.fb1:
        shared["fb1"] = np.asarray(inputs["ff_b1"], f)
    if fl.fb2:
        shared["fb2"] = np.asarray(inputs["ff_b2"], f)
    for nm, key, use in [("g1", "ln1_g", fl.g1), ("b1", "ln1_b", fl.b1),
                         ("g2", "ln2_g", fl.g2), ("b2", "ln2_b", fl.b2),
                         ("g3", "ln3_g", fl.g3), ("b3", "ln3_b", fl.b3)]:
        if use:
            shared[nm] = np.asarray(inputs[key], f)
    if fl.m1 and not fl.m1full:
        # constant triangular mask for the last QT slots, same on all cores:
        # m1c[k', s, q'] = 1 if s*P + k' <= q'
        kk = np.arange(P)[:, None, None]
        ss = np.arange(QT)[None, :, None]
        qq = np.arange(W)[None, None, :]
        shared["m1c"] = ((ss * P + kk) <= qq).astype(bf)

    xTb = [np.ascontiguousarray(x[b].T).astype(bf) for b in range(B)]
    in_maps = []
    for c in range(8):
        b, r = c // 4, c % 4
        m = dict(shared)
        # rotate key tiles: slot t holds physical tile p(t) = (t+(r+1)*QT)%NT
        perm = [(t + (r + 1) * QT) % NT for t in range(NT)]
        xt = xTb[b].reshape(D, NT, P)
        m["xTr"] = np.ascontiguousarray(
            xt[:, perm, :].reshape(D, S))
        if fl.m1 and not fl.m1full:
            zv = np.zeros((P, NT), f)
            for t in range(NT):
                if perm[t] < (r + 1) * QT:
                    zv[:, t] = 1
            m["zv1"] = zv
        if fl.m1full:
            # generic multiplicative mask in rotated key coordinates
            tb = trg[b] if trg.shape[0] == B else trg[0]
            blk = (tb[0, r * W : (r + 1) * W, :] != 0).astype(f)  # [W, S](q,k)
            mk = blk.T.reshape(NT, P, W)  # [kt, k', q]
            m["m1f"] = np.ascontiguousarray(mk[perm]).astype(bf)
        if fl.zv2:
            # this core's own quarter only (keys it contributes to the
            # gathered V2) -- applied before the gather, so consumers see
            # already-zeroed rows
            eb = (np.asarray(enc)[b, 0, 0, r * W : (r + 1) * W] != 0).astype(f)
            m["zv2"] = np.ascontiguousarray(
                eb.reshape(W // P, P).T).astype(f)
        in_maps.append(m)
    return in_maps, fl


def kernel_with_results(**inputs):
    from concourse.bass_utils import run_bass_kernel_spmd

    cfg = Cfg()
    x = np.asarray(inputs["x"])
    assert x.shape == (cfg.B, cfg.S, cfg.D), x.shape
    in_maps, fl = prep_inputs(inputs, cfg)
    nc = make_program(cfg, fl)
    res = run_bass_kernel_spmd(nc, in_maps, list(range(8)))
    y = np.empty((cfg.B, cfg.S, cfg.D), np.float32)
    for c in range(8):
        b, r = c // 4, c % 4
        y[b, r * cfg.W : (r + 1) * cfg.W, :] = res.results[c]["out"].T
    return y, res


def kernel(**inputs):
    return kernel_with_results(**inputs)[0]
